# revision 70
# baseline (speedup 1.0000x reference)
"""MPNN-GGNN forward on 8 Trainium2 NeuronCores.

Data-parallel over the batch: 8 graphs per core. All weights replicated.
Per-core Bass/Tile kernel computes 4 message-passing + GRU steps and the
gated readout entirely on-chip; f32r matmuls at full PE rate, gh in fp8e4
DoubleRow.

v2 (281us -> 239us): the GRU runs in TRANSPOSED (feature-major) layout
with the node dimension PACKED to the 112 real nodes (the reference's
node_on = arange(128) < 112 is structural). Gates are computed as
out[gate_chunk(128), fb*node(448)] with GRU weights as stationary lhsT
([feat, gate] chunks) and mT / hT8 as the moving rhs spanning one
readout free-block (4 graphs x 112 nodes) per instruction. Wins vs v1:
  - GRU biases become per-partition columns -> folded into the Act
    engine's func(x*scale + bias) for free (r/z/u/n, exact f32). Only
    ghn's bias (multiplied by r) stays a fp8-DR matmul plane. Kills 3
    of 4 bias matmuls per gate group.
  - h' emerges feature-major = exactly what the next step's projection
    lhsT and the readout rhs need: all 32 h-transposes and their
    PSUM->SBUF copies disappear.
  - packed-112 free dims cut every gate GEMM / GRU elementwise /
    m-transpose / readout matmul by 12.5%.
  - hT8 is parity-double-buffered (D(s) reads s%2, E(s) writes (s+1)%2)
    since the DR chunk-pairs span all 4 h-chunks.
  - uniform fb1-first schedule: A-phase graph order [4..7,0..3] and
    D/E fb-order [1,0] every step, so D(fb1) only needs mid-A mT's and
    the trailing E(fb0) chain is always covered by the next A-phase
    (no fb-parity-switch bubble); boot DMA carries hT0 for g4/g5.
  - GRU elementwise spread: Act 4 activations, DVE the 2 PSUM-reading
    ops, Pool (idle otherwise) zh/un/h' + the fp8 cast.
  - s3: fb1's readout L0 is injected between fb0's gate-GEMM chunk
    groups; readout layers zipper fb1/fb0 as before.

Layout conventions per core (G = 8 graphs, NR = 112 nodes, H = 512):
  hT_all [128(feat), HC, G*NR] f32r  feature-major hidden state
  hT8_all[128, 2, HC, G*NR] f8       fp8 copy for DoubleRow gh
  mT_all [128, MC, G*NR] f32r        message^T, rhs of the gate GEMMs
  mask_sb[128(w), G, L, 128(v)]      (e^T == l+1) one-hot adjacency
  matmul convention: out[i,j] = sum_k lhsT[k,i] * rhs[k,j]
"""

import numpy as np

import concourse.mybir as mybir
import concourse.tile as tile
from concourse import bacc
from concourse.bass_utils import run_bass_kernel_spmd

# problem constants (hardcoded per contract)
B, N, F_IN = 64, 128, 128
H, MSG, L = 512, 512, 4
NSTEP = 4
TARGET = 12
NCORES = 8
G = B // NCORES          # graphs per core
HC = H // 128            # h chunks
MC = MSG // 128          # msg chunks
GC = 3 * H // 128        # gate chunks (12)
FB = 2                   # readout free blocks (4 graphs x 112 nodes each)
GPB = G // FB
NR = 112                 # real nodes per graph (reference: arange(N) < 112)
NFB = GPB * NR           # packed free size per fb (448)

f32 = mybir.dt.float32
f32r = mybir.dt.float32r
f8 = mybir.dt.float8e4
AF = mybir.ActivationFunctionType
ALU = mybir.AluOpType
AX = mybir.AxisListType
DR = mybir.MatmulPerfMode.DoubleRow
GSC = 16.0  # gate-preact PSUM scale: wihT/whh8T premultiplied by 16
MASK8_DMA = True

_CACHE = {}


def _build(nreps=1):
    nc = bacc.Bacc("TRN2", target_bir_lowering=False)

    # ---- DRAM I/O ----
    # boot: hT0 for graphs 0-1 + A[l=0, hc=0] packed in one early DMA
    d_boot = nc.dram_tensor("boot", [128, 2 * N + MSG], f32r,
                            kind="ExternalInput")
    d_hT0 = nc.dram_tensor("hT0", [F_IN, G, N], f32r, kind="ExternalInput")
    d_mask = nc.dram_tensor("mask", [N, G, L, N],
                            f8 if MASK8_DMA else f32r, kind="ExternalInput")
    d_A = nc.dram_tensor("A", [128, L, HC, MSG], f32r, kind="ExternalInput")
    # transposed GRU weights: [feat, chunk, gate-col]
    d_wihT = nc.dram_tensor("wihT", [128, MC, GC * 128], f32r,
                            kind="ExternalInput")
    d_whh8T = nc.dram_tensor("whh8T", [128, HC, GC * 128], f8,
                             kind="ExternalInput")
    # step-0 lhsT pairs: rz (whh chunk0, 0), ghn (whh_n chunk0, 0)
    d_s0rz8 = nc.dram_tensor("s0rz8", [128, 2, 2 * H], f8,
                             kind="ExternalInput")
    d_s0ghn8 = nc.dram_tensor("s0ghn8", [128, 2, H], f8, kind="ExternalInput")
    # gate bias columns (f32): r(0:4) z(4:8) -z(8:12) bin(12:16)
    # bhn*GSC(16:20) -- the ghn bias rides in the rhn STT, not a matmul
    d_gbias = nc.dram_tensor("gbias", [128, 20], f32, kind="ExternalInput")
    # readout mask broadcast, computed host-side from h_in (sum(h0) != 0)
    d_mb = nc.dram_tensor("mb", [TARGET, G * NR], f32, kind="ExternalInput")
    d_identcol = nc.dram_tensor("identcol", [128, 129], f32r,
                                kind="ExternalInput")
    d_rowb = nc.dram_tensor("rowb", [128, 17, 128], f32r, kind="ExternalInput")
    d_row3 = nc.dram_tensor("row3", [128, 2, TARGET], f32r,
                            kind="ExternalInput")
    d_robias = nc.dram_tensor("robias", [128, 8], f32, kind="ExternalInput")
    d_rob12 = nc.dram_tensor("rob12", [TARGET, 2], f32, kind="ExternalInput")
    d_out = nc.dram_tensor("out", [TARGET, G], f32, kind="ExternalOutput")

    with tile.TileContext(nc) as tc:
        with tc.tile_pool(name="st", bufs=1) as st, \
             tc.tile_pool(name="state", bufs=1) as stt, \
             tc.tile_pool(name="wk", bufs=2) as wk, \
             tc.tile_pool(name="ps", bufs=1, space="PSUM") as ps:

            # ---- static loads, in consumption order ----
            boot_t = st.tile([128, 2 * N + MSG], f32r, tag="boot")
            nc.sync.dma_start(boot_t[:], d_boot[:])
            # p-state prewarm: the PE ramps 0.65->1.2->2.4 GHz over ~3us of
            # continuous busy. Fill the boot-DMA wait with zero matmuls on a
            # memset tile so real work starts at full clock.
            zwarm = st.tile([128, 256], f32r, tag="zwarm")
            nc.gpsimd.memset(zwarm[:].bitcast(f32), 0.0)
            pwarm = ps.tile([128, 256], f32, tag="pP", bufs=2, name="pwarm")
            for wi in range(12):
                nc.tensor.matmul(pwarm[:], zwarm[:, 0:128], zwarm[:],
                                 start=(wi == 0), stop=(wi == 11))
            boot_hT0 = [boot_t[:, 0:N], boot_t[:, N:2 * N]]
            boot_A0 = boot_t[:, 2 * N:2 * N + MSG]
            hT0_sb = st.tile([F_IN, G, N], f32r, tag="hT0")
            A_sb = st.tile([128, L, HC, MSG], f32r, tag="A")
            nc.sync.dma_start(A_sb[:, 1, 0, :], d_A[:, 1, 0, :])
            nc.sync.dma_start(A_sb[:, 2, 0, :], d_A[:, 2, 0, :])
            nc.sync.dma_start(A_sb[:, 3, 0, :], d_A[:, 3, 0, :])
            identcol_t = st.tile([128, 129], f32r, tag="identcol")
            nc.sync.dma_start(identcol_t[:], d_identcol[:])
            ident_sb = identcol_t[:, 0:128]
            onescol_sb = identcol_t[:, 128:129]
            mask_sb = st.tile([N, G, L, N], f32r, tag="mask")
            mask8st = None
            if MASK8_DMA:
                mask8st = st.tile([N, 2, L, N], f8, tag="mask8st")

            def mask_load(g_):
                if MASK8_DMA:
                    sl8 = g_ % 2
                    nc.sync.dma_start(mask8st[:, sl8, :, :],
                                      d_mask[:, g_, :, :])
                    nc.gpsimd.tensor_scalar_mul(mask_sb[:, g_, :, :],
                                                mask8st[:, sl8, :, :], 1.0)
                else:
                    nc.sync.dma_start(mask_sb[:, g_, :, :],
                                      d_mask[:, g_, :, :])

            mask_load(GPB)
            mask_load(GPB + 1)
            nc.sync.dma_start(hT0_sb[:], d_hT0[:])
            nc.sync.dma_start(A_sb[:, 0, 0, :], d_A[:, 0, 0, :])
            for g_ in [GPB + 2, GPB + 3] + list(range(GPB)):
                mask_load(g_)
            wihT_sb = st.tile([128, MC, GC * 128], f32r, tag="wihT")
            for c in range(MC):
                nc.sync.dma_start(wihT_sb[:, c, :], d_wihT[:, c, :])
            gbias_t = st.tile([128, 20], f32, tag="gbias")
            nc.sync.dma_start(gbias_t[:], d_gbias[:])
            s0rz8_t = st.tile([128, 2, 2 * H], f8, tag="s0rz8")
            nc.sync.dma_start(s0rz8_t[:], d_s0rz8[:])
            s0ghn8_t = st.tile([128, 2, H], f8, tag="s0ghn8")
            nc.sync.dma_start(s0ghn8_t[:], d_s0ghn8[:])

            # state tiles
            hT_all = stt.tile([128, HC, G * NR], f32r, tag="hT_all")
            # parity-double-buffered: D(s) reads parity s%2 while E(s)
            # writes parity (s+1)%2 (the DR pairs span all 4 chunks, so a
            # single buffer would RAW-hazard against the per-chunk updates)
            hT8_all = stt.tile([128, 2, HC, G * NR], f8, tag="hT8_all")
            hT08 = stt.tile([128, 2, G * NR], f8, tag="hT08")
            # chunk 1 = zero pair partner for s0's single-chunk DR groups
            nc.gpsimd.memset(hT08[:, 1, :], 0.0)
            for g_ in range(G):
                nc.gpsimd.tensor_scalar_mul(
                    hT08[:, 0, g_ * NR:(g_ + 1) * NR],
                    hT0_sb[:, g_, 0:NR], 1.0)
            mT_all = stt.tile([128, MC, G * NR], f32r, tag="mT_all")

            mb_sb = st.tile([TARGET, G * NR], f32, tag="mb_sb")
            nc.sync.dma_start(mb_sb[:], d_mb[:])
            for hc_ in range(1, HC):
                for l_ in range(L):
                    nc.sync.dma_start(A_sb[:, l_, hc_, :], d_A[:, l_, hc_, :])
            whh8T_sb = st.tile([128, HC, GC * 128], f8, tag="whh8T")
            nc.sync.dma_start(whh8T_sb[:], d_whh8T[:])

            rowb_t = st.tile([128, 17, 128], f32r, tag="rowb")
            nc.sync.dma_start(rowb_t[:], d_rowb[:])
            r1w0_sb = rowb_t[:, 0:5, :]
            r1w1_sb = rowb_t[:, 5:7, :]
            r1w2_sb = rowb_t[:, 7:9, :]
            r2w0_sb = rowb_t[:, 9:13, :]
            r2w1_sb = rowb_t[:, 13:15, :]
            r2w2_sb = rowb_t[:, 15:17, :]
            row3_t = st.tile([128, 2, TARGET], f32r, tag="row3")
            nc.sync.dma_start(row3_t[:], d_row3[:])
            r1w3_sb = row3_t[:, 0, :]
            r2w3_sb = row3_t[:, 1, :]
            robias_t = st.tile([128, 8], f32, tag="robias")
            nc.sync.dma_start(robias_t[:], d_robias[:])
            r1b0_sb = robias_t[:, 0:1]
            r1b1_sb = robias_t[:, 1:3]
            r1b2_sb = robias_t[:, 3:4]
            r2b0_sb = robias_t[:, 4:5]
            r2b1_sb = robias_t[:, 5:7]
            r2b2_sb = robias_t[:, 7:8]
            rob12_t = st.tile([TARGET, 2], f32, tag="rob12")
            nc.sync.dma_start(rob12_t[:], d_rob12[:])
            r1b3_sb = rob12_t[:, 0:1]
            r2b3_sb = rob12_t[:, 1:2]

            for _rep in range(nreps):
                # NOTE: the reference's per-step node_mask multiply is
                # dropped: masked inputs guarantee no edges touch virtual
                # nodes, their per-node GRU lanes never mix into real nodes,
                # and the readout re-applies mask_row.

                # ---- readout (layer-major over 4 independent chains) ----
                out_sb = st.tile([TARGET, G], f32, tag="out_sb")
                r1_ws = [[r1w0_sb[:, kc, :] for kc in range(5)],
                         [r1w1_sb[:, oc, :] for oc in range(2)],
                         [r1w2_sb[:, kc, :] for kc in range(2)],
                         r1w3_sb[:]]
                r1_bs = [r1b0_sb[:],
                         [r1b1_sb[:, oc:oc + 1] for oc in range(2)],
                         r1b2_sb[:]]
                r2_ws = [[r2w0_sb[:, kc, :] for kc in range(4)],
                         [r2w1_sb[:, oc, :] for oc in range(2)],
                         [r2w2_sb[:, kc, :] for kc in range(2)],
                         r2w3_sb[:]]
                r2_bs = [r2b0_sb[:],
                         [r2b1_sb[:, oc:oc + 1] for oc in range(2)],
                         r2b2_sb[:]]
                chains = []
                for fb in range(FB):
                    gsl = slice(fb * GPB, (fb + 1) * GPB)
                    fsl = slice(fb * NFB, (fb + 1) * NFB)
                    h_in_chunks = [hT_all[:, kc, fsl] for kc in range(HC)]
                    chains.append(dict(fb=fb, w="g", ws=r1_ws, bs=r1_bs,
                                       ins=h_in_chunks
                                       + [hT0_sb[:, gsl, 0:NR]]))
                    chains.append(dict(fb=fb, w="v", ws=r2_ws, bs=r2_bs,
                                       ins=h_in_chunks))
                relueng = [None, nc.vector, None, nc.vector]

                def relu_from(dst, src, bias, ci, eng="auto"):
                    if eng == "split":
                        # half-width on Act + DVE in parallel: ~350ns vs
                        # ~600ns for the full tile (tail is relu-latency
                        # bound)
                        hw_ = NFB // 2
                        nc.scalar.activation(dst[:, 0:hw_], src[:, 0:hw_],
                                             AF.Relu, bias=bias)
                        nc.vector.tensor_scalar(dst[:, hw_:], src[:, hw_:],
                                                bias, 0.0,
                                                op0=ALU.add, op1=ALU.max)
                        return
                    if eng == "auto":
                        eng = relueng[ci]
                    if eng is None:
                        nc.scalar.activation(dst, src, AF.Relu, bias=bias)
                    else:
                        eng.tensor_scalar(dst, src, bias, 0.0,
                                          op0=ALU.add, op1=ALU.max)

                def ro_l0(ch, ci, eng="auto"):
                    key = f"{ch['w']}{ch['fb']}"
                    p = ps.tile([128, NFB], f32,
                                tag="pP" if ci % 2 == 0 else "pG2", bufs=2,
                                name=f"rop0_{key}")
                    for i, (wap, rhs) in enumerate(zip(ch["ws"][0], ch["ins"])):
                        nc.tensor.matmul(p[:], wap, rhs, start=(i == 0),
                                         stop=(i == len(ch["ins"]) - 1))
                    a1 = wk.tile([128, NFB], f32r, tag="P", bufs=8,
                                 name=f"roa1_{key}")
                    relu_from(a1[:], p[:], ch["bs"][0], ci, eng)
                    ch["a1"] = a1

                def ro_l1(fb, eng="auto"):
                    for ci0, ch in enumerate(chains[2 * fb:2 * fb + 2]):
                        ci = 2 * fb + ci0
                        key = f"{ch['w']}{ch['fb']}"
                        ch["a2"] = []
                        for oc in range(2):
                            p2 = ps.tile([128, NFB], f32,
                                         tag="pP" if oc == 0 else "pG2",
                                         bufs=2, name=f"rop1_{key}_{oc}")
                            nc.tensor.matmul(p2[:], ch["ws"][1][oc],
                                             ch["a1"][:],
                                             start=True, stop=True)
                            t = wk.tile([128, NFB], f32r, tag="P", bufs=8,
                                        name=f"roa2_{key}_{oc}")
                            relu_from(t[:], p2[:], ch["bs"][1][oc],
                                      (ci + oc) % 2, eng)
                            ch["a2"].append(t)

                def ro_l2(fb, eng="auto"):
                    for ci0, ch in enumerate(chains[2 * fb:2 * fb + 2]):
                        key = f"{ch['w']}{ch['fb']}"
                        p3 = ps.tile([128, NFB], f32,
                                     tag="pP" if ci0 == 0 else "pG2",
                                     bufs=2, name=f"rop2_{key}")
                        for kc in range(2):
                            nc.tensor.matmul(p3[:], ch["ws"][2][kc],
                                             ch["a2"][kc][:],
                                             start=(kc == 0), stop=(kc == 1))
                        a3 = wk.tile([128, NFB], f32r, tag="P", bufs=8,
                                     name=f"roa3_{key}")
                        relu_from(a3[:], p3[:], ch["bs"][2], 0, eng)
                        ch["a3"] = a3

                def ro_l3(fb):
                    for ch in chains[2 * fb:2 * fb + 2]:
                        key = f"{ch['w']}{ch['fb']}"
                        p4 = ps.tile([TARGET, NFB], f32, tag="pGN", bufs=2,
                                     name=f"rop3_{key}")
                        nc.tensor.matmul(p4[:], ch["ws"][3], ch["a3"][:],
                                         start=True, stop=True)
                        ch["p4"] = p4

                def ro_finals(fb):
                    # finals: sum_v gate*val*mask per graph
                    fsl = slice(fb * NFB, (fb + 1) * NFB)
                    chg, chv = chains[2 * fb:2 * fb + 2]
                    vm = wk.tile([TARGET, NFB], f32, tag="z", bufs=2,
                                 name=f"vm_{fb}")
                    nc.vector.scalar_tensor_tensor(
                        vm[:], chv["p4"][:], r2b3_sb[:], mb_sb[:, fsl],
                        op0=ALU.add, op1=ALU.mult)
                    gate_s = wk.tile([TARGET, NFB], f32, tag="r", bufs=2,
                                     name=f"gate_{fb}")
                    nc.scalar.activation(gate_s[:], chg["p4"][:], AF.Sigmoid,
                                         bias=r1b3_sb[:])
                    for gg in range(GPB):
                        ga = fb * GPB + gg
                        sc = wk.tile([TARGET, NR], f32, tag="t1", bufs=2,
                                     name=f"sc_{fb}_{gg}")
                        nc.vector.scalar_tensor_tensor(
                            sc[:], gate_s[:, gg * NR:(gg + 1) * NR], 1.0,
                            vm[:, gg * NR:(gg + 1) * NR],
                            op0=ALU.mult, op1=ALU.mult,
                            accum_out=out_sb[:, ga:ga + 1])

                # ---- message passing steps ----
                for s in range(NSTEP):
                    hcs = [0] if s == 0 else list(range(HC))

                    # -- phase A (per graph): projections + agg -> mT_all --
                    def proj_lhsT(g, hc):
                        if s == 0:
                            assert hc == 0
                            if GPB <= g < GPB + 2:
                                return boot_hT0[g - GPB]
                            return hT0_sb[:, g, :]
                        return hT_all[:, hc, g * NR:(g + 1) * NR]

                    def projections(g):
                        P_sb = []
                        cpeng = [nc.vector.tensor_copy, nc.scalar.copy,
                                 nc.scalar.copy, nc.scalar.copy]
                        if s == 0:
                            cpeng[2] = nc.vector.tensor_copy
                        for l in range(L):
                            # s0: spread projection PSUM across pP + the
                            # (GRU-idle) pG2 pool
                            ptag = "pP" if l % 2 == 0 else "pG2"
                            pp = ps.tile([128, MSG], f32, tag=ptag, bufs=2,
                                         name=f"pp_{s}_{g}_{l}")
                            pv = pp[:] if s == 0 else pp[0:NR, :]
                            if s == 0 and GPB <= g < GPB + 2 and l == 0:
                                nc.tensor.matmul(pv, boot_hT0[g - GPB],
                                                 boot_A0,
                                                 start=True, stop=True)
                            else:
                                for i, hc in enumerate(hcs):
                                    nc.tensor.matmul(pv, proj_lhsT(g, hc),
                                                     A_sb[:, l, hc, :],
                                                     start=(i == 0),
                                                     stop=(i == len(hcs) - 1))
                            psb = wk.tile([128, MSG], f32r, tag="P", bufs=8,
                                          name=f"psb_{s}_{g}_{l}")
                            cpeng[l](psb[:], pp[:])
                            P_sb.append(psb)
                        return P_sb

                    def agg_m(g, P_sb):
                        mp = ps.tile([128, MSG], f32, tag="pMT", bufs=2,
                                     name=f"mp_{s}_{g}")
                        for l in range(L):
                            nc.tensor.matmul(mp[0:NR, :],
                                             mask_sb[:, g, l, 0:NR],
                                             P_sb[l][:],
                                             start=(l == 0), stop=(l == L - 1))
                        m_sb = wk.tile([128, MSG], f32r, tag="m", bufs=4,
                                       name=f"m_{s}_{g}")
                        nc.vector.tensor_copy(m_sb[0:NR, :], mp[0:NR, :])
                        return m_sb

                    def mT_make(g, m_sb):
                        # s0: pGN is free until the D-phase; avoids 3-deep
                        # pMT pressure from the delayed-transpose pipeline
                        tp = ps.tile([128, MC, NR], f32r,
                                     tag="pGN" if s == 0 else "pMT", bufs=2,
                                     name=f"tp_{s}_{g}")
                        for c in range(MC):
                            nc.tensor.transpose(
                                tp[:, c, :],
                                m_sb[0:NR, c * 128:(c + 1) * 128],
                                ident_sb[0:NR, 0:NR])
                        nc.scalar.copy(mT_all[:, :, g * NR:(g + 1) * NR],
                                       tp[:])

                    # fb1's graphs first every step: D(fb1) then needs only
                    # mid-A mT's, and E(s, fb0) consistently trails into the
                    # next A-phase's fb1 half (no fb-parity switch bubble)
                    gorder = list(range(GPB, G)) + list(range(GPB))
                    if s == 0:
                        # short s0 projections expose the agg->copy->
                        # transpose latency: delay each pair's transposes
                        # until after the next pair's projections
                        pending_m = []
                        for gp in range(G // 2):
                            g0, g1 = gorder[2 * gp], gorder[2 * gp + 1]
                            Ps0 = projections(g0)
                            Ps1 = projections(g1)
                            for g_, m_ in pending_m:
                                mT_make(g_, m_)
                            m0 = agg_m(g0, Ps0)
                            m1 = agg_m(g1, Ps1)
                            pending_m = [(g0, m0), (g1, m1)]
                        for g_, m_ in pending_m:
                            mT_make(g_, m_)
                    else:
                        for gp in range(G // 2):
                            g0, g1 = gorder[2 * gp], gorder[2 * gp + 1]
                            Ps0 = projections(g0)
                            Ps1 = projections(g1)
                            mT_make(g0, agg_m(g0, Ps0))
                            mT_make(g1, agg_m(g1, Ps1))

                    # -- phase D+E (per fb, per h-chunk c): gate GEMMs + GRU --
                    def emit_mms(o, mms):
                        for i, (lh, rh, pm) in enumerate(mms):
                            nc.tensor.matmul(o, lh, rh, start=(i == 0),
                                             stop=(i == len(mms) - 1),
                                             perf_mode=pm)

                    fborder = [1, 0]
                    for fb in fborder:
                        fbsl = slice(fb * NFB, (fb + 1) * NFB)
                        for c in range(HC):
                            # gate chunks: r=c, z=4+c, n(i)=8+c, n(h)=8+c
                            r_ps = ps.tile([128, NFB], f32, tag="pP", bufs=2,
                                           name=f"rps_{s}_{fb}_{c}")
                            z_ps = ps.tile([128, NFB], f32, tag="pG2", bufs=2,
                                           name=f"zps_{s}_{fb}_{c}")
                            gin_ps = ps.tile([128, NFB], f32, tag="pGN",
                                             bufs=2, name=f"gin_{s}_{fb}_{c}")
                            ghn_ps = ps.tile([128, NFB], f32, tag="pMT",
                                             bufs=2, name=f"ghn_{s}_{fb}_{c}")
                            # ghn: gh chunks (+ bias plane) only, no wih
                            ghn_mms = []
                            if s == 0:
                                ghn_mms.append((
                                    s0ghn8_t[:, :, c * 128:(c + 1) * 128],
                                    hT08[:, :, fbsl], DR))
                            else:
                                for c2 in (0, 2):
                                    ghn_mms.append((
                                        whh8T_sb[:, c2:c2 + 2,
                                                 (8 + c) * 128:(9 + c) * 128],
                                        hT8_all[:, s % 2, c2:c2 + 2, fbsl],
                                        DR))
                            emit_mms(ghn_ps[:], ghn_mms)

                            def rz_mms(gc):
                                csl = slice(gc * 128, (gc + 1) * 128)
                                mms = []
                                if s == 0:
                                    mms.append((s0rz8_t[:, :, csl],
                                                hT08[:, :, fbsl], DR))
                                else:
                                    for c2 in (0, 2):
                                        mms.append((
                                            whh8T_sb[:, c2:c2 + 2, csl],
                                            hT8_all[:, s % 2, c2:c2 + 2,
                                                    fbsl], DR))
                                for c2 in range(MC):
                                    mms.append((wihT_sb[:, c2, csl],
                                                mT_all[:, c2, fbsl], None))
                                return mms

                            emit_mms(r_ps[:], rz_mms(c))
                            emit_mms(z_ps[:], rz_mms(4 + c))
                            gin_mms = [(wihT_sb[:, c2,
                                                (8 + c) * 128:(9 + c) * 128],
                                        mT_all[:, c2, fbsl],
                                        None) for c2 in range(MC)]
                            emit_mms(gin_ps[:], gin_mms)

                            # -- E: gate nonlinearities + state update --
                            r_sb = wk.tile([128, NFB], f32, tag="r", bufs=2,
                                           name=f"r_{s}_{fb}_{c}")
                            nc.scalar.activation(r_sb[:], r_ps[:], AF.Sigmoid,
                                                 scale=1.0 / GSC,
                                                 bias=gbias_t[:, c:c + 1])
                            if not (s == 0 and c > 0):
                                z_sb = wk.tile([128, NFB], f32, tag="z",
                                               bufs=2, name=f"z_{s}_{fb}_{c}")
                                nc.scalar.activation(
                                    z_sb[:], z_ps[:], AF.Sigmoid,
                                    scale=1.0 / GSC,
                                    bias=gbias_t[:, 4 + c:5 + c])
                            u_sb = wk.tile([128, NFB], f32, tag="u", bufs=2,
                                           name=f"u_{s}_{fb}_{c}")
                            nc.scalar.activation(
                                u_sb[:], z_ps[:], AF.Sigmoid,
                                scale=-1.0 / GSC,
                                bias=gbias_t[:, 8 + c:9 + c])
                            # rhn16 = (ghn + 16*bhn)*r = 16*r*h_n; npre =
                            # gin + rhn16 = 16*(i_n - bin + r*h_n); the 1/16
                            # folds into tanh's scale, bin into its bias
                            rhn = wk.tile([128, NFB], f32, tag="t1", bufs=2,
                                          name=f"rhn_{s}_{fb}_{c}")
                            nc.vector.scalar_tensor_tensor(
                                rhn[:], ghn_ps[:],
                                gbias_t[:, 16 + c:17 + c], r_sb[:],
                                op0=ALU.add, op1=ALU.mult)
                            npre = wk.tile([128, NFB], f32, tag="t2", bufs=2,
                                           name=f"npre_{s}_{fb}_{c}")
                            nc.vector.tensor_add(npre[:], gin_ps[:], rhn[:])
                            n_sb = wk.tile([128, NFB], f32, tag="n", bufs=2,
                                           name=f"n_{s}_{fb}_{c}")
                            nc.scalar.activation(n_sb[:], npre[:], AF.Tanh,
                                                 scale=1.0 / GSC,
                                                 bias=gbias_t[:, 12 + c:13 + c])
                            # h' = (1-z)*n + z*h. zh is off the critical path
                            # (ready before n) -> Pool; un/h'add gate the
                            # next step's proj lhsT -> keep on DVE
                            hsl = hT_all[:, c, fbsl]
                            if s == 0:
                                if c == 0:
                                    zh = wk.tile([128, NFB], f32, tag="zh",
                                                 bufs=2, name=f"zh_{s}_{fb}")
                                    nc.gpsimd.tensor_mul(
                                        zh[:], z_sb[:],
                                        hT0_sb[:, fb * GPB:(fb + 1) * GPB,
                                               0:NR])
                                    un = wk.tile([128, NFB], f32, tag="un",
                                                 bufs=2, name=f"un_{s}_{fb}")
                                    nc.gpsimd.tensor_mul(un[:], u_sb[:],
                                                         n_sb[:])
                                    nc.gpsimd.tensor_add(hsl, un[:], zh[:])
                                else:
                                    nc.gpsimd.tensor_mul(hsl, u_sb[:],
                                                         n_sb[:])
                            else:
                                zh = wk.tile([128, NFB], f32, tag="zh",
                                             bufs=2, name=f"zh_{s}_{fb}_{c}")
                                nc.gpsimd.tensor_mul(zh[:], z_sb[:], hsl)
                                un = wk.tile([128, NFB], f32, tag="un",
                                             bufs=2, name=f"un_{s}_{fb}_{c}")
                                eng_un = (nc.gpsimd if s == NSTEP - 1
                                          else nc.vector)
                                eng_un.tensor_mul(un[:], u_sb[:], n_sb[:])
                                eng_un.tensor_add(hsl, un[:], zh[:])
                            if s < NSTEP - 1:
                                nc.gpsimd.tensor_scalar_mul(
                                    hT8_all[:, (s + 1) % 2, c, fbsl],
                                    hsl, 1.0)
                            if s == NSTEP - 1 and fb == fborder[-1] and c >= 2:
                                # fb1's hT is done (its D/E ran first): start
                                # its readout L0 between fb0's D chunks so
                                # the relus queue ahead of fb0's E tail
                                ro_l0(chains[2 + (c - 2)], 2 + (c - 2))

                        if s == NSTEP - 1 and fb == fborder[-1]:
                            # fb1's hT is complete (its D/E ran first):
                            # overlap its readout L0..L2 with fb0's E tail
                            ro_l1(1)
                            ro_l2(1)

                # zippered readout, fb1 first
                ro_l0(chains[0], 0)
                ro_l0(chains[1], 1)
                ro_l3(1)
                ro_l1(0)
                ro_finals(1)
                ro_l2(0)
                ro_l3(0)
                ro_finals(0)
                nc.sync.dma_start(d_out[:], out_sb[:])

    nc.compile()
    return nc


def _prep_core_inputs(core, g_, h_in, e):
    cs = slice(core * G, (core + 1) * G)
    f = np.float32
    hT0 = np.ascontiguousarray(h_in[cs].transpose(2, 0, 1))  # [F, G, N]
    labels = np.arange(1, L + 1, dtype=f)
    # mask[w, g, l, v] = (e[g, v, w] == l+1)
    e_c = e[cs]  # [G, V, W]
    oh = (e_c[:, None, :, :] == labels[None, :, None, None]).astype(f)
    mask = np.ascontiguousarray(oh.transpose(3, 0, 1, 2))  # [W, G, L, V]
    if MASK8_DMA:
        import ml_dtypes
        mask = mask.astype(ml_dtypes.float8_e4m3)
    # readout mask broadcast (reference: sum(h0, -1) != 0), packed to 112
    rmask = (h_in[cs].sum(-1) != 0).astype(f)[:, 0:NR]      # [G, NR]
    mb = np.broadcast_to(rmask.reshape(1, G * NR),
                         (TARGET, G * NR)).copy()
    return {
        "hT0": hT0,
        "mask": mask,
        "mb": mb,
    }


def _prep_shared_inputs(A, gru_Wih, gru_Whh, gru_bih, gru_bhh,
                        r1_Ws, r1_bs, r2_Ws, r2_bs):
    f = np.float32

    def chunk_rows(M, nch):  # [K, C] -> [128, nch, C] with K = nch*128
        K, C = M.shape
        assert K == nch * 128
        return np.ascontiguousarray(M.reshape(nch, 128, C).transpose(1, 0, 2))

    import ml_dtypes
    f8np = ml_dtypes.float8_e4m3
    GSCf = np.float32(GSC)

    A_t = np.ascontiguousarray(
        A.reshape(L, HC, 128, MSG).transpose(2, 0, 1, 3))  # [128, L, HC, MSG]

    # transposed GRU weights: [feat, chunk, gate-col], premultiplied by GSC
    def t_weights(W, nch):  # W [3H, K] -> [128(f), nch, GC*128]
        Wt = (W * GSCf).reshape(GC, 128, nch, 128)  # [gc, j, kc, f]
        return np.ascontiguousarray(
            Wt.transpose(3, 2, 0, 1).reshape(128, nch, GC * 128))

    wihT = t_weights(np.asarray(gru_Wih, f), MC)
    whh8T_f = t_weights(np.asarray(gru_Whh, f), HC)
    whh8T = whh8T_f.astype(f8np)

    bih = np.asarray(gru_bih, f)
    bhh = np.asarray(gru_bhh, f)
    brz = (bih + bhh)[:2 * H]
    bin_ = bih[2 * H:]
    bhn = bhh[2 * H:]
    # gbias columns: r(0:4) z(4:8) -z(8:12) bin(12:16) bhn*GSC(16:20)
    gbias = np.zeros((128, 20), f)
    for c in range(4):
        gbias[:, c] = brz[c * 128:(c + 1) * 128]
        gbias[:, 4 + c] = brz[H + c * 128:H + (c + 1) * 128]
        gbias[:, 8 + c] = -brz[H + c * 128:H + (c + 1) * 128]
        gbias[:, 12 + c] = bin_[c * 128:(c + 1) * 128]
        gbias[:, 16 + c] = GSCf * bhn[c * 128:(c + 1) * 128]
    # step-0 lhsT pairs
    s0rz8 = np.zeros((128, 2, 2 * H), f)
    s0rz8[:, 0, :] = whh8T_f[:, 0, 0:2 * H]
    s0ghn8 = np.zeros((128, 2, H), f)
    s0ghn8[:, 0, :] = whh8T_f[:, 0, 2 * H:3 * H]

    # readout weights, transposed layout
    r1w0t = np.ascontiguousarray(r1_Ws[0].T)  # [2H, 128]
    r1w0 = np.zeros((128, 5, 128), f)
    for kc in range(4):
        r1w0[:, kc, :] = r1w0t[kc * 128:(kc + 1) * 128]
    r1w0[:, 4, :] = r1w0t[H:H + F_IN]  # h0 chunk (features 0:128 of h0 half)
    r1w1 = np.ascontiguousarray(r1_Ws[1].T.reshape(128, 2, 128))
    r1w2 = chunk_rows(np.ascontiguousarray(r1_Ws[2].T), 2)
    r1w3 = np.ascontiguousarray(r1_Ws[3].T)  # [128, 12]
    r2w0 = chunk_rows(np.ascontiguousarray(r2_Ws[0].T), 4)
    r2w1 = np.ascontiguousarray(r2_Ws[1].T.reshape(128, 2, 128))
    r2w2 = chunk_rows(np.ascontiguousarray(r2_Ws[2].T), 2)
    r2w3 = np.ascontiguousarray(r2_Ws[3].T)

    identcol = np.concatenate([np.eye(128, dtype=f), np.ones((128, 1), f)], 1)
    rowb = np.concatenate([r1w0, r1w1, r1w2, r2w0, r2w1, r2w2], axis=1)
    row3 = np.stack([r1w3, r2w3], axis=1)
    robias = np.concatenate([
        r1_bs[0].reshape(-1, 1).astype(f),
        np.ascontiguousarray(r1_bs[1].reshape(2, 128).T),
        r1_bs[2].reshape(-1, 1).astype(f),
        r2_bs[0].reshape(-1, 1).astype(f),
        np.ascontiguousarray(r2_bs[1].reshape(2, 128).T),
        r2_bs[2].reshape(-1, 1).astype(f)], axis=1)
    rob12 = np.concatenate([r1_bs[3].reshape(-1, 1).astype(f),
                            r2_bs[3].reshape(-1, 1).astype(f)], axis=1)
    return {
        "A": A_t,
        "wihT": wihT,
        "whh8T": np.ascontiguousarray(whh8T),
        "s0rz8": s0rz8.astype(f8np),
        "s0ghn8": s0ghn8.astype(f8np),
        "gbias": gbias,
        "identcol": np.ascontiguousarray(identcol),
        "rowb": np.ascontiguousarray(rowb),
        "row3": np.ascontiguousarray(row3),
        "robias": np.ascontiguousarray(robias),
        "rob12": np.ascontiguousarray(rob12),
    }


def _get_nc(nreps=1):
    key = ("nc", nreps)
    if key not in _CACHE:
        _CACHE[key] = _build(nreps)
    return _CACHE[key]


def _run(in_maps, **kwargs):
    nc = _get_nc()
    return run_bass_kernel_spmd(nc, in_maps, core_ids=list(range(NCORES)),
                                **kwargs)


def make_in_maps(g, h_in, e, A, gru_Wih, gru_Whh, gru_bih, gru_bhh,
                 r1_W0, r1_b0, r1_W1, r1_b1, r1_W2, r1_b2, r1_W3, r1_b3,
                 r2_W0, r2_b0, r2_W1, r2_b1, r2_W2, r2_b2, r2_W3, r2_b3):
    r1_Ws, r1_bs = [r1_W0, r1_W1, r1_W2, r1_W3], [r1_b0, r1_b1, r1_b2, r1_b3]
    r2_Ws, r2_bs = [r2_W0, r2_W1, r2_W2, r2_W3], [r2_b0, r2_b1, r2_b2, r2_b3]
    arrs = {k: np.asarray(v, np.float32) for k, v in dict(
        g=g, h_in=h_in, e=e, A=A, gru_Wih=gru_Wih, gru_Whh=gru_Whh,
        gru_bih=gru_bih, gru_bhh=gru_bhh).items()}
    r1_Ws = [np.asarray(w, np.float32) for w in r1_Ws]
    r1_bs = [np.asarray(b, np.float32) for b in r1_bs]
    r2_Ws = [np.asarray(w, np.float32) for w in r2_Ws]
    r2_bs = [np.asarray(b, np.float32) for b in r2_bs]
    shared = _prep_shared_inputs(arrs["A"], arrs["gru_Wih"], arrs["gru_Whh"],
                                 arrs["gru_bih"], arrs["gru_bhh"],
                                 r1_Ws, r1_bs, r2_Ws, r2_bs)
    f = np.float32
    in_maps = []
    for core in range(NCORES):
        m = dict(shared)
        m.update(_prep_core_inputs(core, arrs["g"], arrs["h_in"], arrs["e"]))
        boot = np.concatenate([m["hT0"][:, GPB, :], m["hT0"][:, GPB + 1, :],
                               np.asarray(arrs["A"][0, 0:128, :], f)], 1)
        m["boot"] = np.ascontiguousarray(boot)
        in_maps.append(m)
    return in_maps


def kernel(**inputs):
    in_maps = make_in_maps(**inputs)
    res = _run(in_maps)
    out = np.zeros((B, TARGET), np.float32)
    for core in range(NCORES):
        out[core * G:(core + 1) * G] = res.results[core]["out"].T
    return out


if __name__ == "__main__":
    import reference
    inputs = {k: np.asarray(v) for k, v in reference.setup_inputs().items()}
    expected = np.asarray(reference.reference(**inputs))
    actual = kernel(**inputs)
    scale = np.abs(expected).max()
    err = np.abs(actual - expected).max() / scale
    print("Relative error:", err)


# revision 75
# speedup vs baseline: 1.0090x; 1.0090x over previous
"""MPNN-GGNN forward on 8 Trainium2 NeuronCores.

Data-parallel over the batch: 8 graphs per core. All weights replicated.
Per-core Bass/Tile kernel computes 4 message-passing + GRU steps and the
gated readout entirely on-chip; f32r matmuls at full PE rate, gh in fp8e4
DoubleRow.

v2 (281us -> 235us): the GRU runs in TRANSPOSED (feature-major) layout
with the node dimension PACKED to the 112 real nodes (the reference's
node_on = arange(128) < 112 is structural). Gates are computed as
out[gate_chunk(128), fb*node(448)] with GRU weights as stationary lhsT
([feat, gate] chunks) and mT / hT8 as the moving rhs spanning one
readout free-block (4 graphs x 112 nodes) per instruction. Wins vs v1:
  - GRU biases become per-partition columns, all exact f32 and all
    free: r/z/u/bin via the Act engine's func(x*scale + bias), bhn via
    the rhn STT's scalar-AP slot ((ghn + 16*bhn)*r, with the 1/16
    rescale folded into tanh's scale). Zero bias matmuls remain.
  - h' emerges feature-major = exactly what the next step's projection
    lhsT and the readout rhs need: all 32 h-transposes and their
    PSUM->SBUF copies disappear.
  - packed-112 free dims cut every gate GEMM / GRU elementwise /
    m-transpose / readout matmul by 12.5%.
  - hT8 is parity-double-buffered (D(s) reads s%2, E(s) writes (s+1)%2)
    since the DR chunk-pairs span all 4 h-chunks.
  - uniform fb1-first schedule: A-phase graph order [4..7,0..3] and
    D/E fb-order [1,0] every step, so D(fb1) only needs mid-A mT's and
    the trailing E(fb0) chain is always covered by the next A-phase
    (no fb-parity-switch bubble); boot DMA carries hT0 for g4/g5.
  - GRU elementwise spread: Act 4 activations, DVE the 2 PSUM-reading
    ops, Pool (idle otherwise) zh/un/h' + the fp8 cast.
  - s3: fb1's readout L0 is injected between fb0's gate-GEMM chunk
    groups; readout layers zipper fb1/fb0 as before.

Layout conventions per core (G = 8 graphs, NR = 112 nodes, H = 512):
  hT_all [128(feat), HC, G*NR] f32r  feature-major hidden state
  hT8_all[128, 2, HC, G*NR] f8       fp8 copy for DoubleRow gh
  mT_all [128, MC, G*NR] f32r        message^T, rhs of the gate GEMMs
  mask_sb[128(w), G, L, 128(v)]      (e^T == l+1) one-hot adjacency
  matmul convention: out[i,j] = sum_k lhsT[k,i] * rhs[k,j]
"""

import numpy as np

import concourse.mybir as mybir
import concourse.tile as tile
from concourse import bacc
from concourse.bass_utils import run_bass_kernel_spmd

# problem constants (hardcoded per contract)
B, N, F_IN = 64, 128, 128
H, MSG, L = 512, 512, 4
NSTEP = 4
TARGET = 12
NCORES = 8
G = B // NCORES          # graphs per core
HC = H // 128            # h chunks
MC = MSG // 128          # msg chunks
GC = 3 * H // 128        # gate chunks (12)
FB = 2                   # readout free blocks (4 graphs x 112 nodes each)
GPB = G // FB
NR = 112                 # real nodes per graph (reference: arange(N) < 112)
NFB = GPB * NR           # packed free size per fb (448)

f32 = mybir.dt.float32
f32r = mybir.dt.float32r
f16 = mybir.dt.float16
f8 = mybir.dt.float8e4
AF = mybir.ActivationFunctionType
ALU = mybir.AluOpType
AX = mybir.AxisListType
DR = mybir.MatmulPerfMode.DoubleRow
GSC = 16.0  # gate-preact PSUM scale: wihT/whh8T premultiplied by 16
MASK8_DMA = True

_CACHE = {}


def _build(nreps=1):
    nc = bacc.Bacc("TRN2", target_bir_lowering=False)

    # ---- DRAM I/O ----
    # boot: hT0 for graphs 0-1 + A[l=0, hc=0] packed in one early DMA
    d_boot = nc.dram_tensor("boot", [128, 2 * N + MSG], f32r,
                            kind="ExternalInput")
    d_hT0 = nc.dram_tensor("hT0", [F_IN, G, N], f32r, kind="ExternalInput")
    d_mask = nc.dram_tensor("mask", [N, G, L, N],
                            f8 if MASK8_DMA else f32r, kind="ExternalInput")
    d_A = nc.dram_tensor("A", [128, L, HC, MSG], f32r, kind="ExternalInput")
    # transposed GRU weights: [feat, chunk, gate-col]
    d_wihT = nc.dram_tensor("wihT", [128, MC, GC * 128], f32r,
                            kind="ExternalInput")
    d_whh8T = nc.dram_tensor("whh8T", [128, HC, GC * 128], f8,
                             kind="ExternalInput")
    # step-0 lhsT pairs: rz (whh chunk0, 0), ghn (whh_n chunk0, 0)
    d_s0rz8 = nc.dram_tensor("s0rz8", [128, 2, 2 * H], f8,
                             kind="ExternalInput")
    d_s0ghn8 = nc.dram_tensor("s0ghn8", [128, 2, H], f8, kind="ExternalInput")
    # gate bias columns (f32): r(0:4) z(4:8) -z(8:12) bin(12:16)
    # bhn*GSC(16:20) -- the ghn bias rides in the rhn STT, not a matmul
    d_gbias = nc.dram_tensor("gbias", [128, 20], f32, kind="ExternalInput")
    # readout mask broadcast, computed host-side from h_in (sum(h0) != 0)
    d_mb = nc.dram_tensor("mb", [TARGET, G * NR], f32, kind="ExternalInput")
    d_identcol = nc.dram_tensor("identcol", [128, 129], f32r,
                                kind="ExternalInput")
    d_rowb = nc.dram_tensor("rowb", [128, 17, 128], f32r, kind="ExternalInput")
    d_row3 = nc.dram_tensor("row3", [128, 2, TARGET], f32r,
                            kind="ExternalInput")
    d_robias = nc.dram_tensor("robias", [128, 8], f32, kind="ExternalInput")
    d_rob12 = nc.dram_tensor("rob12", [TARGET, 2], f32, kind="ExternalInput")
    d_out = nc.dram_tensor("out", [TARGET, G], f32, kind="ExternalOutput")

    with tile.TileContext(nc) as tc:
        with tc.tile_pool(name="st", bufs=1) as st, \
             tc.tile_pool(name="state", bufs=1) as stt, \
             tc.tile_pool(name="wk", bufs=2) as wk, \
             tc.tile_pool(name="ps", bufs=1, space="PSUM") as ps:

            # ---- static loads, in consumption order ----
            boot_t = st.tile([128, 2 * N + MSG], f32r, tag="boot")
            nc.sync.dma_start(boot_t[:], d_boot[:])
            # p-state prewarm: the PE ramps 0.65->1.2->2.4 GHz over ~3us of
            # continuous busy. Fill the boot-DMA wait with zero matmuls on a
            # memset tile so real work starts at full clock.
            zwarm = st.tile([128, 256], f32r, tag="zwarm")
            nc.gpsimd.memset(zwarm[:].bitcast(f32), 0.0)
            pwarm = ps.tile([128, 256], f32, tag="pP", bufs=2, name="pwarm")
            for wi in range(12):
                nc.tensor.matmul(pwarm[:], zwarm[:, 0:128], zwarm[:],
                                 start=(wi == 0), stop=(wi == 11))
            boot_hT0 = [boot_t[:, 0:N], boot_t[:, N:2 * N]]
            boot_A0 = boot_t[:, 2 * N:2 * N + MSG]
            hT0_sb = st.tile([F_IN, G, N], f32r, tag="hT0")
            A_sb = st.tile([128, L, HC, MSG], f32r, tag="A")
            nc.sync.dma_start(A_sb[:, 1, 0, :], d_A[:, 1, 0, :])
            nc.sync.dma_start(A_sb[:, 2, 0, :], d_A[:, 2, 0, :])
            nc.sync.dma_start(A_sb[:, 3, 0, :], d_A[:, 3, 0, :])
            identcol_t = st.tile([128, 129], f32r, tag="identcol")
            nc.sync.dma_start(identcol_t[:], d_identcol[:])
            ident_sb = identcol_t[:, 0:128]
            onescol_sb = identcol_t[:, 128:129]
            # fp16 identity: m transposes run at 1.0 cyc/row (vs 1.5 f32r);
            # m is rounded to fp16 (2^-11, measured 0.006 standalone err)
            ident16_t = st.tile([128, 128], f16, tag="ident16")
            nc.gpsimd.tensor_scalar_mul(ident16_t[:], ident_sb, 1.0)
            mask_sb = st.tile([N, G, L, N], f32r, tag="mask")
            mask8st = None
            if MASK8_DMA:
                mask8st = st.tile([N, 2, L, N], f8, tag="mask8st")

            def mask_load(g_):
                if MASK8_DMA:
                    sl8 = g_ % 2
                    nc.sync.dma_start(mask8st[:, sl8, :, :],
                                      d_mask[:, g_, :, :])
                    nc.gpsimd.tensor_scalar_mul(mask_sb[:, g_, :, :],
                                                mask8st[:, sl8, :, :], 1.0)
                else:
                    nc.sync.dma_start(mask_sb[:, g_, :, :],
                                      d_mask[:, g_, :, :])

            mask_load(GPB)
            mask_load(GPB + 1)
            nc.sync.dma_start(hT0_sb[:], d_hT0[:])
            nc.sync.dma_start(A_sb[:, 0, 0, :], d_A[:, 0, 0, :])
            for g_ in [GPB + 2, GPB + 3] + list(range(GPB)):
                mask_load(g_)
            wihT_sb = st.tile([128, MC, GC * 128], f32r, tag="wihT")
            for c in range(MC):
                nc.sync.dma_start(wihT_sb[:, c, :], d_wihT[:, c, :])
            gbias_t = st.tile([128, 20], f32, tag="gbias")
            nc.sync.dma_start(gbias_t[:], d_gbias[:])
            s0rz8_t = st.tile([128, 2, 2 * H], f8, tag="s0rz8")
            nc.sync.dma_start(s0rz8_t[:], d_s0rz8[:])
            s0ghn8_t = st.tile([128, 2, H], f8, tag="s0ghn8")
            nc.sync.dma_start(s0ghn8_t[:], d_s0ghn8[:])

            # state tiles
            hT_all = stt.tile([128, HC, G * NR], f32r, tag="hT_all")
            # parity-double-buffered: D(s) reads parity s%2 while E(s)
            # writes parity (s+1)%2 (the DR pairs span all 4 chunks, so a
            # single buffer would RAW-hazard against the per-chunk updates)
            hT8_all = stt.tile([128, 2, HC, G * NR], f8, tag="hT8_all")
            hT08 = stt.tile([128, 2, G * NR], f8, tag="hT08")
            # chunk 1 = zero pair partner for s0's single-chunk DR groups
            nc.gpsimd.memset(hT08[:, 1, :], 0.0)
            for g_ in range(G):
                nc.gpsimd.tensor_scalar_mul(
                    hT08[:, 0, g_ * NR:(g_ + 1) * NR],
                    hT0_sb[:, g_, 0:NR], 1.0)
            mT_all = stt.tile([128, MC, G * NR], f32r, tag="mT_all")

            mb_sb = st.tile([TARGET, G * NR], f32, tag="mb_sb")
            nc.sync.dma_start(mb_sb[:], d_mb[:])
            for hc_ in range(1, HC):
                for l_ in range(L):
                    nc.sync.dma_start(A_sb[:, l_, hc_, :], d_A[:, l_, hc_, :])
            whh8T_sb = st.tile([128, HC, GC * 128], f8, tag="whh8T")
            nc.sync.dma_start(whh8T_sb[:], d_whh8T[:])

            rowb_t = st.tile([128, 17, 128], f32r, tag="rowb")
            nc.sync.dma_start(rowb_t[:], d_rowb[:])
            r1w0_sb = rowb_t[:, 0:5, :]
            r1w1_sb = rowb_t[:, 5:7, :]
            r1w2_sb = rowb_t[:, 7:9, :]
            r2w0_sb = rowb_t[:, 9:13, :]
            r2w1_sb = rowb_t[:, 13:15, :]
            r2w2_sb = rowb_t[:, 15:17, :]
            row3_t = st.tile([128, 2, TARGET], f32r, tag="row3")
            nc.sync.dma_start(row3_t[:], d_row3[:])
            r1w3_sb = row3_t[:, 0, :]
            r2w3_sb = row3_t[:, 1, :]
            robias_t = st.tile([128, 8], f32, tag="robias")
            nc.sync.dma_start(robias_t[:], d_robias[:])
            r1b0_sb = robias_t[:, 0:1]
            r1b1_sb = robias_t[:, 1:3]
            r1b2_sb = robias_t[:, 3:4]
            r2b0_sb = robias_t[:, 4:5]
            r2b1_sb = robias_t[:, 5:7]
            r2b2_sb = robias_t[:, 7:8]
            rob12_t = st.tile([TARGET, 2], f32, tag="rob12")
            nc.sync.dma_start(rob12_t[:], d_rob12[:])
            r1b3_sb = rob12_t[:, 0:1]
            r2b3_sb = rob12_t[:, 1:2]

            for _rep in range(nreps):
                # NOTE: the reference's per-step node_mask multiply is
                # dropped: masked inputs guarantee no edges touch virtual
                # nodes, their per-node GRU lanes never mix into real nodes,
                # and the readout re-applies mask_row.

                # ---- readout (layer-major over 4 independent chains) ----
                out_sb = st.tile([TARGET, G], f32, tag="out_sb")
                r1_ws = [[r1w0_sb[:, kc, :] for kc in range(5)],
                         [r1w1_sb[:, oc, :] for oc in range(2)],
                         [r1w2_sb[:, kc, :] for kc in range(2)],
                         r1w3_sb[:]]
                r1_bs = [r1b0_sb[:],
                         [r1b1_sb[:, oc:oc + 1] for oc in range(2)],
                         r1b2_sb[:]]
                r2_ws = [[r2w0_sb[:, kc, :] for kc in range(4)],
                         [r2w1_sb[:, oc, :] for oc in range(2)],
                         [r2w2_sb[:, kc, :] for kc in range(2)],
                         r2w3_sb[:]]
                r2_bs = [r2b0_sb[:],
                         [r2b1_sb[:, oc:oc + 1] for oc in range(2)],
                         r2b2_sb[:]]
                chains = []
                for fb in range(FB):
                    gsl = slice(fb * GPB, (fb + 1) * GPB)
                    fsl = slice(fb * NFB, (fb + 1) * NFB)
                    h_in_chunks = [hT_all[:, kc, fsl] for kc in range(HC)]
                    chains.append(dict(fb=fb, w="g", ws=r1_ws, bs=r1_bs,
                                       ins=h_in_chunks
                                       + [hT0_sb[:, gsl, 0:NR]]))
                    chains.append(dict(fb=fb, w="v", ws=r2_ws, bs=r2_bs,
                                       ins=h_in_chunks))
                relueng = [None, nc.vector, None, nc.vector]

                def relu_from(dst, src, bias, ci, eng="auto"):
                    if eng == "split":
                        # half-width on Act + DVE in parallel: ~350ns vs
                        # ~600ns for the full tile (tail is relu-latency
                        # bound)
                        hw_ = NFB // 2
                        nc.scalar.activation(dst[:, 0:hw_], src[:, 0:hw_],
                                             AF.Relu, bias=bias)
                        nc.vector.tensor_scalar(dst[:, hw_:], src[:, hw_:],
                                                bias, 0.0,
                                                op0=ALU.add, op1=ALU.max)
                        return
                    if eng == "auto":
                        eng = relueng[ci]
                    if eng is None:
                        nc.scalar.activation(dst, src, AF.Relu, bias=bias)
                    else:
                        eng.tensor_scalar(dst, src, bias, 0.0,
                                          op0=ALU.add, op1=ALU.max)

                def ro_l0(ch, ci, eng="auto"):
                    key = f"{ch['w']}{ch['fb']}"
                    p = ps.tile([128, NFB], f32,
                                tag="pP" if ci % 2 == 0 else "pG2", bufs=2,
                                name=f"rop0_{key}")
                    for i, (wap, rhs) in enumerate(zip(ch["ws"][0], ch["ins"])):
                        nc.tensor.matmul(p[:], wap, rhs, start=(i == 0),
                                         stop=(i == len(ch["ins"]) - 1))
                    a1 = wk.tile([128, NFB], f32r, tag="P", bufs=8,
                                 name=f"roa1_{key}")
                    relu_from(a1[:], p[:], ch["bs"][0], ci, eng)
                    ch["a1"] = a1

                def ro_l1(fb, eng="auto"):
                    for ci0, ch in enumerate(chains[2 * fb:2 * fb + 2]):
                        ci = 2 * fb + ci0
                        key = f"{ch['w']}{ch['fb']}"
                        ch["a2"] = []
                        for oc in range(2):
                            p2 = ps.tile([128, NFB], f32,
                                         tag="pP" if oc == 0 else "pG2",
                                         bufs=2, name=f"rop1_{key}_{oc}")
                            nc.tensor.matmul(p2[:], ch["ws"][1][oc],
                                             ch["a1"][:],
                                             start=True, stop=True)
                            t = wk.tile([128, NFB], f32r, tag="P", bufs=8,
                                        name=f"roa2_{key}_{oc}")
                            relu_from(t[:], p2[:], ch["bs"][1][oc],
                                      (ci + oc) % 2, eng)
                            ch["a2"].append(t)

                def ro_l2(fb, eng="auto"):
                    for ci0, ch in enumerate(chains[2 * fb:2 * fb + 2]):
                        key = f"{ch['w']}{ch['fb']}"
                        p3 = ps.tile([128, NFB], f32,
                                     tag="pP" if ci0 == 0 else "pG2",
                                     bufs=2, name=f"rop2_{key}")
                        for kc in range(2):
                            nc.tensor.matmul(p3[:], ch["ws"][2][kc],
                                             ch["a2"][kc][:],
                                             start=(kc == 0), stop=(kc == 1))
                        a3 = wk.tile([128, NFB], f32r, tag="P", bufs=8,
                                     name=f"roa3_{key}")
                        relu_from(a3[:], p3[:], ch["bs"][2], 0, eng)
                        ch["a3"] = a3

                def ro_l3(fb):
                    for ch in chains[2 * fb:2 * fb + 2]:
                        key = f"{ch['w']}{ch['fb']}"
                        p4 = ps.tile([TARGET, NFB], f32, tag="pGN", bufs=2,
                                     name=f"rop3_{key}")
                        nc.tensor.matmul(p4[:], ch["ws"][3], ch["a3"][:],
                                         start=True, stop=True)
                        ch["p4"] = p4

                def ro_finals(fb):
                    # finals: sum_v gate*val*mask per graph
                    fsl = slice(fb * NFB, (fb + 1) * NFB)
                    chg, chv = chains[2 * fb:2 * fb + 2]
                    vm = wk.tile([TARGET, NFB], f32, tag="z", bufs=2,
                                 name=f"vm_{fb}")
                    nc.vector.scalar_tensor_tensor(
                        vm[:], chv["p4"][:], r2b3_sb[:], mb_sb[:, fsl],
                        op0=ALU.add, op1=ALU.mult)
                    gate_s = wk.tile([TARGET, NFB], f32, tag="r", bufs=2,
                                     name=f"gate_{fb}")
                    nc.scalar.activation(gate_s[:], chg["p4"][:], AF.Sigmoid,
                                         bias=r1b3_sb[:])
                    for gg in range(GPB):
                        ga = fb * GPB + gg
                        sc = wk.tile([TARGET, NR], f32, tag="t1", bufs=2,
                                     name=f"sc_{fb}_{gg}")
                        nc.vector.scalar_tensor_tensor(
                            sc[:], gate_s[:, gg * NR:(gg + 1) * NR], 1.0,
                            vm[:, gg * NR:(gg + 1) * NR],
                            op0=ALU.mult, op1=ALU.mult,
                            accum_out=out_sb[:, ga:ga + 1])

                # ---- message passing steps ----
                for s in range(NSTEP):
                    hcs = [0] if s == 0 else list(range(HC))

                    # -- phase A (per graph): projections + agg -> mT_all --
                    def proj_lhsT(g, hc):
                        if s == 0:
                            assert hc == 0
                            if GPB <= g < GPB + 2:
                                return boot_hT0[g - GPB]
                            return hT0_sb[:, g, :]
                        return hT_all[:, hc, g * NR:(g + 1) * NR]

                    def projections(g):
                        P_sb = []
                        cpeng = [nc.vector.tensor_copy, nc.scalar.copy,
                                 nc.scalar.copy, nc.scalar.copy]
                        if s == 0:
                            cpeng[2] = nc.vector.tensor_copy
                        for l in range(L):
                            # s0: spread projection PSUM across pP + the
                            # (GRU-idle) pG2 pool
                            ptag = "pP" if l % 2 == 0 else "pG2"
                            pp = ps.tile([128, MSG], f32, tag=ptag, bufs=2,
                                         name=f"pp_{s}_{g}_{l}")
                            pv = pp[:] if s == 0 else pp[0:NR, :]
                            if s == 0 and GPB <= g < GPB + 2 and l == 0:
                                nc.tensor.matmul(pv, boot_hT0[g - GPB],
                                                 boot_A0,
                                                 start=True, stop=True)
                            else:
                                for i, hc in enumerate(hcs):
                                    nc.tensor.matmul(pv, proj_lhsT(g, hc),
                                                     A_sb[:, l, hc, :],
                                                     start=(i == 0),
                                                     stop=(i == len(hcs) - 1))
                            psb = wk.tile([128, MSG], f32r, tag="P", bufs=8,
                                          name=f"psb_{s}_{g}_{l}")
                            cpeng[l](psb[:], pp[:])
                            P_sb.append(psb)
                        return P_sb

                    def agg_m(g, P_sb):
                        mp = ps.tile([128, MSG], f32, tag="pMT", bufs=2,
                                     name=f"mp_{s}_{g}")
                        for l in range(L):
                            nc.tensor.matmul(mp[0:NR, :],
                                             mask_sb[:, g, l, 0:NR],
                                             P_sb[l][:],
                                             start=(l == 0), stop=(l == L - 1))
                        m_sb = wk.tile([128, MSG], f16, tag="m", bufs=4,
                                       name=f"m_{s}_{g}")
                        nc.vector.tensor_copy(m_sb[0:NR, :], mp[0:NR, :])
                        return m_sb

                    def mT_make(g, m_sb):
                        # s0: pGN is free until the D-phase; avoids 3-deep
                        # pMT pressure from the delayed-transpose pipeline
                        tp = ps.tile([128, MC, NR], f16,
                                     tag="pGN" if s == 0 else "pMT", bufs=2,
                                     name=f"tp_{s}_{g}")
                        for c in range(MC):
                            nc.tensor.transpose(
                                tp[:, c, :],
                                m_sb[0:NR, c * 128:(c + 1) * 128],
                                ident16_t[0:NR, 0:NR])
                        nc.scalar.copy(mT_all[:, :, g * NR:(g + 1) * NR],
                                       tp[:])

                    # fb1's graphs first every step: D(fb1) then needs only
                    # mid-A mT's, and E(s, fb0) consistently trails into the
                    # next A-phase's fb1 half (no fb-parity switch bubble)
                    gorder = list(range(GPB, G)) + list(range(GPB))
                    if s == 0:
                        # short s0 projections expose the agg->copy->
                        # transpose latency: delay each pair's transposes
                        # until after the next pair's projections
                        pending_m = []
                        for gp in range(G // 2):
                            g0, g1 = gorder[2 * gp], gorder[2 * gp + 1]
                            Ps0 = projections(g0)
                            Ps1 = projections(g1)
                            for g_, m_ in pending_m:
                                mT_make(g_, m_)
                            m0 = agg_m(g0, Ps0)
                            m1 = agg_m(g1, Ps1)
                            pending_m = [(g0, m0), (g1, m1)]
                        for g_, m_ in pending_m:
                            mT_make(g_, m_)
                    else:
                        for gp in range(G // 2):
                            g0, g1 = gorder[2 * gp], gorder[2 * gp + 1]
                            Ps0 = projections(g0)
                            Ps1 = projections(g1)
                            mT_make(g0, agg_m(g0, Ps0))
                            mT_make(g1, agg_m(g1, Ps1))

                    # -- phase D+E (per fb, per h-chunk c): gate GEMMs + GRU --
                    def emit_mms(o, mms):
                        for i, (lh, rh, pm) in enumerate(mms):
                            nc.tensor.matmul(o, lh, rh, start=(i == 0),
                                             stop=(i == len(mms) - 1),
                                             perf_mode=pm)

                    fborder = [1, 0]
                    for fb in fborder:
                        fbsl = slice(fb * NFB, (fb + 1) * NFB)
                        for c in range(HC):
                            # gate chunks: r=c, z=4+c, n(i)=8+c, n(h)=8+c
                            r_ps = ps.tile([128, NFB], f32, tag="pP", bufs=2,
                                           name=f"rps_{s}_{fb}_{c}")
                            z_ps = ps.tile([128, NFB], f32, tag="pG2", bufs=2,
                                           name=f"zps_{s}_{fb}_{c}")
                            gin_ps = ps.tile([128, NFB], f32, tag="pGN",
                                             bufs=2, name=f"gin_{s}_{fb}_{c}")
                            ghn_ps = ps.tile([128, NFB], f32, tag="pMT",
                                             bufs=2, name=f"ghn_{s}_{fb}_{c}")
                            # ghn: gh chunks (+ bias plane) only, no wih
                            ghn_mms = []
                            if s == 0:
                                ghn_mms.append((
                                    s0ghn8_t[:, :, c * 128:(c + 1) * 128],
                                    hT08[:, :, fbsl], DR))
                            else:
                                for c2 in (0, 2):
                                    ghn_mms.append((
                                        whh8T_sb[:, c2:c2 + 2,
                                                 (8 + c) * 128:(9 + c) * 128],
                                        hT8_all[:, s % 2, c2:c2 + 2, fbsl],
                                        DR))
                            emit_mms(ghn_ps[:], ghn_mms)

                            def rz_mms(gc):
                                csl = slice(gc * 128, (gc + 1) * 128)
                                mms = []
                                if s == 0:
                                    mms.append((s0rz8_t[:, :, csl],
                                                hT08[:, :, fbsl], DR))
                                else:
                                    for c2 in (0, 2):
                                        mms.append((
                                            whh8T_sb[:, c2:c2 + 2, csl],
                                            hT8_all[:, s % 2, c2:c2 + 2,
                                                    fbsl], DR))
                                for c2 in range(MC):
                                    mms.append((wihT_sb[:, c2, csl],
                                                mT_all[:, c2, fbsl], None))
                                return mms

                            emit_mms(r_ps[:], rz_mms(c))
                            emit_mms(z_ps[:], rz_mms(4 + c))
                            gin_mms = [(wihT_sb[:, c2,
                                                (8 + c) * 128:(9 + c) * 128],
                                        mT_all[:, c2, fbsl],
                                        None) for c2 in range(MC)]
                            emit_mms(gin_ps[:], gin_mms)

                            # -- E: gate nonlinearities + state update --
                            r_sb = wk.tile([128, NFB], f32, tag="r", bufs=2,
                                           name=f"r_{s}_{fb}_{c}")
                            nc.scalar.activation(r_sb[:], r_ps[:], AF.Sigmoid,
                                                 scale=1.0 / GSC,
                                                 bias=gbias_t[:, c:c + 1])
                            if not (s == 0 and c > 0):
                                z_sb = wk.tile([128, NFB], f32, tag="z",
                                               bufs=2, name=f"z_{s}_{fb}_{c}")
                                nc.scalar.activation(
                                    z_sb[:], z_ps[:], AF.Sigmoid,
                                    scale=1.0 / GSC,
                                    bias=gbias_t[:, 4 + c:5 + c])
                            u_sb = wk.tile([128, NFB], f32, tag="u", bufs=2,
                                           name=f"u_{s}_{fb}_{c}")
                            nc.scalar.activation(
                                u_sb[:], z_ps[:], AF.Sigmoid,
                                scale=-1.0 / GSC,
                                bias=gbias_t[:, 8 + c:9 + c])
                            # rhn16 = (ghn + 16*bhn)*r = 16*r*h_n; npre =
                            # gin + rhn16 = 16*(i_n - bin + r*h_n); the 1/16
                            # folds into tanh's scale, bin into its bias
                            rhn = wk.tile([128, NFB], f32, tag="t1", bufs=2,
                                          name=f"rhn_{s}_{fb}_{c}")
                            nc.vector.scalar_tensor_tensor(
                                rhn[:], ghn_ps[:],
                                gbias_t[:, 16 + c:17 + c], r_sb[:],
                                op0=ALU.add, op1=ALU.mult)
                            npre = wk.tile([128, NFB], f32, tag="t2", bufs=2,
                                           name=f"npre_{s}_{fb}_{c}")
                            nc.vector.tensor_add(npre[:], gin_ps[:], rhn[:])
                            n_sb = wk.tile([128, NFB], f32, tag="n", bufs=2,
                                           name=f"n_{s}_{fb}_{c}")
                            nc.scalar.activation(n_sb[:], npre[:], AF.Tanh,
                                                 scale=1.0 / GSC,
                                                 bias=gbias_t[:, 12 + c:13 + c])
                            # h' = (1-z)*n + z*h. zh is off the critical path
                            # (ready before n) -> Pool; un/h'add gate the
                            # next step's proj lhsT -> keep on DVE
                            hsl = hT_all[:, c, fbsl]
                            if s == 0:
                                if c == 0:
                                    zh = wk.tile([128, NFB], f32, tag="zh",
                                                 bufs=2, name=f"zh_{s}_{fb}")
                                    nc.gpsimd.tensor_mul(
                                        zh[:], z_sb[:],
                                        hT0_sb[:, fb * GPB:(fb + 1) * GPB,
                                               0:NR])
                                    un = wk.tile([128, NFB], f32, tag="un",
                                                 bufs=2, name=f"un_{s}_{fb}")
                                    nc.gpsimd.tensor_mul(un[:], u_sb[:],
                                                         n_sb[:])
                                    nc.gpsimd.tensor_add(hsl, un[:], zh[:])
                                else:
                                    nc.gpsimd.tensor_mul(hsl, u_sb[:],
                                                         n_sb[:])
                            else:
                                zh = wk.tile([128, NFB], f32, tag="zh",
                                             bufs=2, name=f"zh_{s}_{fb}_{c}")
                                nc.gpsimd.tensor_mul(zh[:], z_sb[:], hsl)
                                un = wk.tile([128, NFB], f32, tag="un",
                                             bufs=2, name=f"un_{s}_{fb}_{c}")
                                eng_un = (nc.gpsimd if s == NSTEP - 1
                                          else nc.vector)
                                eng_un.tensor_mul(un[:], u_sb[:], n_sb[:])
                                eng_un.tensor_add(hsl, un[:], zh[:])
                            if s < NSTEP - 1:
                                nc.gpsimd.tensor_scalar_mul(
                                    hT8_all[:, (s + 1) % 2, c, fbsl],
                                    hsl, 1.0)
                            if s == NSTEP - 1 and fb == fborder[-1] and c >= 2:
                                # fb1's hT is done (its D/E ran first): start
                                # its readout L0 between fb0's D chunks so
                                # the relus queue ahead of fb0's E tail
                                ro_l0(chains[2 + (c - 2)], 2 + (c - 2))

                        if s == NSTEP - 1 and fb == fborder[-1]:
                            # fb1's hT is complete (its D/E ran first):
                            # overlap its readout L0..L2 with fb0's E tail
                            ro_l1(1)
                            ro_l2(1)

                # zippered readout, fb1 first
                ro_l0(chains[0], 0)
                ro_l0(chains[1], 1)
                ro_l3(1)
                ro_l1(0)
                ro_finals(1)
                ro_l2(0)
                ro_l3(0)
                ro_finals(0)
                nc.sync.dma_start(d_out[:], out_sb[:])

    nc.compile()
    return nc


def _prep_core_inputs(core, g_, h_in, e):
    cs = slice(core * G, (core + 1) * G)
    f = np.float32
    hT0 = np.ascontiguousarray(h_in[cs].transpose(2, 0, 1))  # [F, G, N]
    labels = np.arange(1, L + 1, dtype=f)
    # mask[w, g, l, v] = (e[g, v, w] == l+1)
    e_c = e[cs]  # [G, V, W]
    oh = (e_c[:, None, :, :] == labels[None, :, None, None]).astype(f)
    mask = np.ascontiguousarray(oh.transpose(3, 0, 1, 2))  # [W, G, L, V]
    if MASK8_DMA:
        import ml_dtypes
        mask = mask.astype(ml_dtypes.float8_e4m3)
    # readout mask broadcast (reference: sum(h0, -1) != 0), packed to 112
    rmask = (h_in[cs].sum(-1) != 0).astype(f)[:, 0:NR]      # [G, NR]
    mb = np.broadcast_to(rmask.reshape(1, G * NR),
                         (TARGET, G * NR)).copy()
    return {
        "hT0": hT0,
        "mask": mask,
        "mb": mb,
    }


def _prep_shared_inputs(A, gru_Wih, gru_Whh, gru_bih, gru_bhh,
                        r1_Ws, r1_bs, r2_Ws, r2_bs):
    f = np.float32

    def chunk_rows(M, nch):  # [K, C] -> [128, nch, C] with K = nch*128
        K, C = M.shape
        assert K == nch * 128
        return np.ascontiguousarray(M.reshape(nch, 128, C).transpose(1, 0, 2))

    import ml_dtypes
    f8np = ml_dtypes.float8_e4m3
    GSCf = np.float32(GSC)

    A_t = np.ascontiguousarray(
        A.reshape(L, HC, 128, MSG).transpose(2, 0, 1, 3))  # [128, L, HC, MSG]

    # transposed GRU weights: [feat, chunk, gate-col], premultiplied by GSC
    def t_weights(W, nch):  # W [3H, K] -> [128(f), nch, GC*128]
        Wt = (W * GSCf).reshape(GC, 128, nch, 128)  # [gc, j, kc, f]
        return np.ascontiguousarray(
            Wt.transpose(3, 2, 0, 1).reshape(128, nch, GC * 128))

    wihT = t_weights(np.asarray(gru_Wih, f), MC)
    whh8T_f = t_weights(np.asarray(gru_Whh, f), HC)
    whh8T = whh8T_f.astype(f8np)

    bih = np.asarray(gru_bih, f)
    bhh = np.asarray(gru_bhh, f)
    brz = (bih + bhh)[:2 * H]
    bin_ = bih[2 * H:]
    bhn = bhh[2 * H:]
    # gbias columns: r(0:4) z(4:8) -z(8:12) bin(12:16) bhn*GSC(16:20)
    gbias = np.zeros((128, 20), f)
    for c in range(4):
        gbias[:, c] = brz[c * 128:(c + 1) * 128]
        gbias[:, 4 + c] = brz[H + c * 128:H + (c + 1) * 128]
        gbias[:, 8 + c] = -brz[H + c * 128:H + (c + 1) * 128]
        gbias[:, 12 + c] = bin_[c * 128:(c + 1) * 128]
        gbias[:, 16 + c] = GSCf * bhn[c * 128:(c + 1) * 128]
    # step-0 lhsT pairs
    s0rz8 = np.zeros((128, 2, 2 * H), f)
    s0rz8[:, 0, :] = whh8T_f[:, 0, 0:2 * H]
    s0ghn8 = np.zeros((128, 2, H), f)
    s0ghn8[:, 0, :] = whh8T_f[:, 0, 2 * H:3 * H]

    # readout weights, transposed layout
    r1w0t = np.ascontiguousarray(r1_Ws[0].T)  # [2H, 128]
    r1w0 = np.zeros((128, 5, 128), f)
    for kc in range(4):
        r1w0[:, kc, :] = r1w0t[kc * 128:(kc + 1) * 128]
    r1w0[:, 4, :] = r1w0t[H:H + F_IN]  # h0 chunk (features 0:128 of h0 half)
    r1w1 = np.ascontiguousarray(r1_Ws[1].T.reshape(128, 2, 128))
    r1w2 = chunk_rows(np.ascontiguousarray(r1_Ws[2].T), 2)
    r1w3 = np.ascontiguousarray(r1_Ws[3].T)  # [128, 12]
    r2w0 = chunk_rows(np.ascontiguousarray(r2_Ws[0].T), 4)
    r2w1 = np.ascontiguousarray(r2_Ws[1].T.reshape(128, 2, 128))
    r2w2 = chunk_rows(np.ascontiguousarray(r2_Ws[2].T), 2)
    r2w3 = np.ascontiguousarray(r2_Ws[3].T)

    identcol = np.concatenate([np.eye(128, dtype=f), np.ones((128, 1), f)], 1)
    rowb = np.concatenate([r1w0, r1w1, r1w2, r2w0, r2w1, r2w2], axis=1)
    row3 = np.stack([r1w3, r2w3], axis=1)
    robias = np.concatenate([
        r1_bs[0].reshape(-1, 1).astype(f),
        np.ascontiguousarray(r1_bs[1].reshape(2, 128).T),
        r1_bs[2].reshape(-1, 1).astype(f),
        r2_bs[0].reshape(-1, 1).astype(f),
        np.ascontiguousarray(r2_bs[1].reshape(2, 128).T),
        r2_bs[2].reshape(-1, 1).astype(f)], axis=1)
    rob12 = np.concatenate([r1_bs[3].reshape(-1, 1).astype(f),
                            r2_bs[3].reshape(-1, 1).astype(f)], axis=1)
    return {
        "A": A_t,
        "wihT": wihT,
        "whh8T": np.ascontiguousarray(whh8T),
        "s0rz8": s0rz8.astype(f8np),
        "s0ghn8": s0ghn8.astype(f8np),
        "gbias": gbias,
        "identcol": np.ascontiguousarray(identcol),
        "rowb": np.ascontiguousarray(rowb),
        "row3": np.ascontiguousarray(row3),
        "robias": np.ascontiguousarray(robias),
        "rob12": np.ascontiguousarray(rob12),
    }


def _get_nc(nreps=1):
    key = ("nc", nreps)
    if key not in _CACHE:
        _CACHE[key] = _build(nreps)
    return _CACHE[key]


def _run(in_maps, **kwargs):
    nc = _get_nc()
    return run_bass_kernel_spmd(nc, in_maps, core_ids=list(range(NCORES)),
                                **kwargs)


def make_in_maps(g, h_in, e, A, gru_Wih, gru_Whh, gru_bih, gru_bhh,
                 r1_W0, r1_b0, r1_W1, r1_b1, r1_W2, r1_b2, r1_W3, r1_b3,
                 r2_W0, r2_b0, r2_W1, r2_b1, r2_W2, r2_b2, r2_W3, r2_b3):
    r1_Ws, r1_bs = [r1_W0, r1_W1, r1_W2, r1_W3], [r1_b0, r1_b1, r1_b2, r1_b3]
    r2_Ws, r2_bs = [r2_W0, r2_W1, r2_W2, r2_W3], [r2_b0, r2_b1, r2_b2, r2_b3]
    arrs = {k: np.asarray(v, np.float32) for k, v in dict(
        g=g, h_in=h_in, e=e, A=A, gru_Wih=gru_Wih, gru_Whh=gru_Whh,
        gru_bih=gru_bih, gru_bhh=gru_bhh).items()}
    r1_Ws = [np.asarray(w, np.float32) for w in r1_Ws]
    r1_bs = [np.asarray(b, np.float32) for b in r1_bs]
    r2_Ws = [np.asarray(w, np.float32) for w in r2_Ws]
    r2_bs = [np.asarray(b, np.float32) for b in r2_bs]
    shared = _prep_shared_inputs(arrs["A"], arrs["gru_Wih"], arrs["gru_Whh"],
                                 arrs["gru_bih"], arrs["gru_bhh"],
                                 r1_Ws, r1_bs, r2_Ws, r2_bs)
    f = np.float32
    in_maps = []
    for core in range(NCORES):
        m = dict(shared)
        m.update(_prep_core_inputs(core, arrs["g"], arrs["h_in"], arrs["e"]))
        boot = np.concatenate([m["hT0"][:, GPB, :], m["hT0"][:, GPB + 1, :],
                               np.asarray(arrs["A"][0, 0:128, :], f)], 1)
        m["boot"] = np.ascontiguousarray(boot)
        in_maps.append(m)
    return in_maps


def kernel(**inputs):
    in_maps = make_in_maps(**inputs)
    res = _run(in_maps)
    out = np.zeros((B, TARGET), np.float32)
    for core in range(NCORES):
        out[core * G:(core + 1) * G] = res.results[core]["out"].T
    return out


if __name__ == "__main__":
    import reference
    inputs = {k: np.asarray(v) for k, v in reference.setup_inputs().items()}
    expected = np.asarray(reference.reference(**inputs))
    actual = kernel(**inputs)
    scale = np.abs(expected).max()
    err = np.abs(actual - expected).max() / scale
    print("Relative error:", err)


# revision 80
# speedup vs baseline: 1.0109x; 1.0019x over previous
"""MPNN-GGNN forward on 8 Trainium2 NeuronCores.

Data-parallel over the batch: 8 graphs per core. All weights replicated.
Per-core Bass/Tile kernel computes 4 message-passing + GRU steps and the
gated readout entirely on-chip; f32r matmuls at full PE rate, gh in fp8e4
DoubleRow.

v2 (281us -> 233us): the GRU runs in TRANSPOSED (feature-major) layout
with the node dimension PACKED to the 112 real nodes (the reference's
node_on = arange(128) < 112 is structural). Gates are computed as
out[gate_chunk(128), fb*node(448)] with GRU weights as stationary lhsT
([feat, gate] chunks) and mT / hT8 as the moving rhs spanning one
readout free-block (4 graphs x 112 nodes) per instruction. Wins vs v1:
  - GRU biases become per-partition columns, all exact f32 and all
    free: r/z/u/bin via the Act engine's func(x*scale + bias), bhn via
    the rhn STT's scalar-AP slot ((ghn + 16*bhn)*r, with the 1/16
    rescale folded into tanh's scale). Zero bias matmuls remain.
  - h' emerges feature-major = exactly what the next step's projection
    lhsT and the readout rhs need: all 32 h-transposes and their
    PSUM->SBUF copies disappear.
  - packed-112 free dims cut every gate GEMM / GRU elementwise /
    m-transpose / readout matmul by 12.5%.
  - m is rounded to fp16 before its PE transpose (1.0 vs 1.5 cyc/row;
    fp16's 2^-11 step measured 0.006 standalone err vs bf16's fatal
    0.029); mT upconverts back to f32r exactly in the PSUM->SBUF copy.
  - hT8 is parity-double-buffered (D(s) reads s%2, E(s) writes (s+1)%2)
    since the DR chunk-pairs span all 4 h-chunks.
  - uniform fb1-first schedule: A-phase graph order [4..7,0..3] and
    D/E fb-order [1,0] every step, so D(fb1) only needs mid-A mT's and
    the trailing E(fb0) chain is always covered by the next A-phase
    (no fb-parity-switch bubble); boot DMA carries hT0 for g4/g5.
  - GRU elementwise spread: Act 4 activations, DVE the 2 PSUM-reading
    ops, Pool (idle otherwise) zh/un/h' + the fp8 cast.
  - s3: fb1's readout L0 is injected between fb0's gate-GEMM chunk
    groups; readout layers zipper fb1/fb0 as before.

Layout conventions per core (G = 8 graphs, NR = 112 nodes, H = 512):
  hT_all [128(feat), HC, G*NR] f32r  feature-major hidden state
  hT8_all[128, 2, HC, G*NR] f8       fp8 copy for DoubleRow gh
  mT_all [128, MC, G*NR] f32r        message^T, rhs of the gate GEMMs
  mask_sb[128(w), G, L, 128(v)]      (e^T == l+1) one-hot adjacency
  matmul convention: out[i,j] = sum_k lhsT[k,i] * rhs[k,j]
"""

import numpy as np

import concourse.mybir as mybir
import concourse.tile as tile
from concourse import bacc
from concourse.bass_utils import run_bass_kernel_spmd

# problem constants (hardcoded per contract)
B, N, F_IN = 64, 128, 128
H, MSG, L = 512, 512, 4
NSTEP = 4
TARGET = 12
NCORES = 8
G = B // NCORES          # graphs per core
HC = H // 128            # h chunks
MC = MSG // 128          # msg chunks
GC = 3 * H // 128        # gate chunks (12)
FB = 2                   # readout free blocks (4 graphs x 112 nodes each)
GPB = G // FB
NR = 112                 # real nodes per graph (reference: arange(N) < 112)
NFB = GPB * NR           # packed free size per fb (448)

f32 = mybir.dt.float32
f32r = mybir.dt.float32r
f16 = mybir.dt.float16
f8 = mybir.dt.float8e4
AF = mybir.ActivationFunctionType
ALU = mybir.AluOpType
AX = mybir.AxisListType
DR = mybir.MatmulPerfMode.DoubleRow
GSC = 16.0  # gate-preact PSUM scale: wihT/whh8T premultiplied by 16
MASK8_DMA = True

_CACHE = {}


def _build(nreps=1):
    nc = bacc.Bacc("TRN2", target_bir_lowering=False)

    # ---- DRAM I/O ----
    # boot: hT0 for graphs 0-1 + A[l=0, hc=0] packed in one early DMA
    d_boot = nc.dram_tensor("boot", [128, 2 * N + MSG], f32r,
                            kind="ExternalInput")
    d_hT0 = nc.dram_tensor("hT0", [F_IN, G, N], f32r, kind="ExternalInput")
    d_mask = nc.dram_tensor("mask", [N, G, L, N],
                            f8 if MASK8_DMA else f32r, kind="ExternalInput")
    d_A = nc.dram_tensor("A", [128, L, HC, MSG], f32r, kind="ExternalInput")
    # transposed GRU weights: [feat, chunk, gate-col]
    d_wihT = nc.dram_tensor("wihT", [128, MC, GC * 128], f32r,
                            kind="ExternalInput")
    d_whh8T = nc.dram_tensor("whh8T", [128, HC, GC * 128], f8,
                             kind="ExternalInput")
    # step-0 lhsT pairs: rz (whh chunk0, 0), ghn (whh_n chunk0, 0)
    d_s0rz8 = nc.dram_tensor("s0rz8", [128, 2, 2 * H], f8,
                             kind="ExternalInput")
    d_s0ghn8 = nc.dram_tensor("s0ghn8", [128, 2, H], f8, kind="ExternalInput")
    # gate bias columns (f32): r(0:4) z(4:8) -z(8:12) bin(12:16)
    # bhn*GSC(16:20) -- the ghn bias rides in the rhn STT, not a matmul
    d_gbias = nc.dram_tensor("gbias", [128, 20], f32, kind="ExternalInput")
    # readout mask broadcast, computed host-side from h_in (sum(h0) != 0)
    d_mb = nc.dram_tensor("mb", [TARGET, G * NR], f32, kind="ExternalInput")
    d_identcol = nc.dram_tensor("identcol", [128, 129], f32r,
                                kind="ExternalInput")
    d_rowb = nc.dram_tensor("rowb", [128, 17, 128], f32r, kind="ExternalInput")
    d_row3 = nc.dram_tensor("row3", [128, 2, TARGET], f32r,
                            kind="ExternalInput")
    d_robias = nc.dram_tensor("robias", [128, 8], f32, kind="ExternalInput")
    d_rob12 = nc.dram_tensor("rob12", [TARGET, 2], f32, kind="ExternalInput")
    d_out = nc.dram_tensor("out", [TARGET, G], f32, kind="ExternalOutput")

    with tile.TileContext(nc) as tc:
        with tc.tile_pool(name="st", bufs=1) as st, \
             tc.tile_pool(name="state", bufs=1) as stt, \
             tc.tile_pool(name="wk", bufs=2) as wk, \
             tc.tile_pool(name="ps", bufs=1, space="PSUM") as ps:

            # ---- static loads, in consumption order ----
            boot_t = st.tile([128, 2 * N + MSG], f32r, tag="boot")
            nc.sync.dma_start(boot_t[:], d_boot[:])
            # p-state prewarm: the PE ramps 0.65->1.2->2.4 GHz over ~3us of
            # continuous busy. Fill the boot-DMA wait with zero matmuls on a
            # memset tile so real work starts at full clock.
            zwarm = st.tile([128, 256], f32r, tag="zwarm")
            nc.gpsimd.memset(zwarm[:].bitcast(f32), 0.0)
            pwarm = ps.tile([128, 256], f32, tag="pP", bufs=2, name="pwarm")
            for wi in range(12):
                nc.tensor.matmul(pwarm[:], zwarm[:, 0:128], zwarm[:],
                                 start=(wi == 0), stop=(wi == 11))
            boot_hT0 = [boot_t[:, 0:N], boot_t[:, N:2 * N]]
            boot_A0 = boot_t[:, 2 * N:2 * N + MSG]
            hT0_sb = st.tile([F_IN, G, N], f32r, tag="hT0")
            A_sb = st.tile([128, L, HC, MSG], f32r, tag="A")
            nc.sync.dma_start(A_sb[:, 1, 0, :], d_A[:, 1, 0, :])
            nc.sync.dma_start(A_sb[:, 2, 0, :], d_A[:, 2, 0, :])
            nc.sync.dma_start(A_sb[:, 3, 0, :], d_A[:, 3, 0, :])
            identcol_t = st.tile([128, 129], f32r, tag="identcol")
            nc.sync.dma_start(identcol_t[:], d_identcol[:])
            ident_sb = identcol_t[:, 0:128]
            onescol_sb = identcol_t[:, 128:129]
            # fp16 identity: m transposes run at 1.0 cyc/row (vs 1.5 f32r);
            # m is rounded to fp16 (2^-11, measured 0.006 standalone err)
            ident16_t = st.tile([128, 128], f16, tag="ident16")
            nc.gpsimd.tensor_scalar_mul(ident16_t[:], ident_sb, 1.0)
            mask_sb = st.tile([N, G, L, N], f32r, tag="mask")
            mask8st = None
            if MASK8_DMA:
                mask8st = st.tile([N, 2, L, N], f8, tag="mask8st")

            def mask_load(g_):
                if MASK8_DMA:
                    sl8 = g_ % 2
                    nc.sync.dma_start(mask8st[:, sl8, :, :],
                                      d_mask[:, g_, :, :])
                    nc.gpsimd.tensor_scalar_mul(mask_sb[:, g_, :, :],
                                                mask8st[:, sl8, :, :], 1.0)
                else:
                    nc.sync.dma_start(mask_sb[:, g_, :, :],
                                      d_mask[:, g_, :, :])

            mask_load(GPB)
            mask_load(GPB + 1)
            nc.sync.dma_start(hT0_sb[:], d_hT0[:])
            nc.sync.dma_start(A_sb[:, 0, 0, :], d_A[:, 0, 0, :])
            for g_ in [GPB + 2, GPB + 3] + list(range(GPB)):
                mask_load(g_)
            wihT_sb = st.tile([128, MC, GC * 128], f32r, tag="wihT")
            for c in range(MC):
                nc.sync.dma_start(wihT_sb[:, c, :], d_wihT[:, c, :])
            gbias_t = st.tile([128, 20], f32, tag="gbias")
            nc.sync.dma_start(gbias_t[:], d_gbias[:])
            s0rz8_t = st.tile([128, 2, 2 * H], f8, tag="s0rz8")
            nc.sync.dma_start(s0rz8_t[:], d_s0rz8[:])
            s0ghn8_t = st.tile([128, 2, H], f8, tag="s0ghn8")
            nc.sync.dma_start(s0ghn8_t[:], d_s0ghn8[:])

            # state tiles
            hT_all = stt.tile([128, HC, G * NR], f32r, tag="hT_all")
            # parity-double-buffered: D(s) reads parity s%2 while E(s)
            # writes parity (s+1)%2 (the DR pairs span all 4 chunks, so a
            # single buffer would RAW-hazard against the per-chunk updates)
            hT8_all = stt.tile([128, 2, HC, G * NR], f8, tag="hT8_all")
            hT08 = stt.tile([128, 2, G * NR], f8, tag="hT08")
            # chunk 1 = zero pair partner for s0's single-chunk DR groups
            nc.gpsimd.memset(hT08[:, 1, :], 0.0)
            for g_ in range(G):
                nc.gpsimd.tensor_scalar_mul(
                    hT08[:, 0, g_ * NR:(g_ + 1) * NR],
                    hT0_sb[:, g_, 0:NR], 1.0)
            mT_all = stt.tile([128, MC, G * NR], f32r, tag="mT_all")

            mb_sb = st.tile([TARGET, G * NR], f32, tag="mb_sb")
            nc.sync.dma_start(mb_sb[:], d_mb[:])
            for hc_ in range(1, HC):
                for l_ in range(L):
                    nc.sync.dma_start(A_sb[:, l_, hc_, :], d_A[:, l_, hc_, :])
            whh8T_sb = st.tile([128, HC, GC * 128], f8, tag="whh8T")
            nc.sync.dma_start(whh8T_sb[:], d_whh8T[:])

            rowb_t = st.tile([128, 17, 128], f32r, tag="rowb")
            nc.sync.dma_start(rowb_t[:], d_rowb[:])
            r1w0_sb = rowb_t[:, 0:5, :]
            r1w1_sb = rowb_t[:, 5:7, :]
            r1w2_sb = rowb_t[:, 7:9, :]
            r2w0_sb = rowb_t[:, 9:13, :]
            r2w1_sb = rowb_t[:, 13:15, :]
            r2w2_sb = rowb_t[:, 15:17, :]
            row3_t = st.tile([128, 2, TARGET], f32r, tag="row3")
            nc.sync.dma_start(row3_t[:], d_row3[:])
            r1w3_sb = row3_t[:, 0, :]
            r2w3_sb = row3_t[:, 1, :]
            robias_t = st.tile([128, 8], f32, tag="robias")
            nc.sync.dma_start(robias_t[:], d_robias[:])
            r1b0_sb = robias_t[:, 0:1]
            r1b1_sb = robias_t[:, 1:3]
            r1b2_sb = robias_t[:, 3:4]
            r2b0_sb = robias_t[:, 4:5]
            r2b1_sb = robias_t[:, 5:7]
            r2b2_sb = robias_t[:, 7:8]
            rob12_t = st.tile([TARGET, 2], f32, tag="rob12")
            nc.sync.dma_start(rob12_t[:], d_rob12[:])
            r1b3_sb = rob12_t[:, 0:1]
            r2b3_sb = rob12_t[:, 1:2]

            for _rep in range(nreps):
                # NOTE: the reference's per-step node_mask multiply is
                # dropped: masked inputs guarantee no edges touch virtual
                # nodes, their per-node GRU lanes never mix into real nodes,
                # and the readout re-applies mask_row.

                # ---- readout (layer-major over 4 independent chains) ----
                out_sb = st.tile([TARGET, G], f32, tag="out_sb")
                r1_ws = [[r1w0_sb[:, kc, :] for kc in range(5)],
                         [r1w1_sb[:, oc, :] for oc in range(2)],
                         [r1w2_sb[:, kc, :] for kc in range(2)],
                         r1w3_sb[:]]
                r1_bs = [r1b0_sb[:],
                         [r1b1_sb[:, oc:oc + 1] for oc in range(2)],
                         r1b2_sb[:]]
                r2_ws = [[r2w0_sb[:, kc, :] for kc in range(4)],
                         [r2w1_sb[:, oc, :] for oc in range(2)],
                         [r2w2_sb[:, kc, :] for kc in range(2)],
                         r2w3_sb[:]]
                r2_bs = [r2b0_sb[:],
                         [r2b1_sb[:, oc:oc + 1] for oc in range(2)],
                         r2b2_sb[:]]
                chains = []
                for fb in range(FB):
                    gsl = slice(fb * GPB, (fb + 1) * GPB)
                    fsl = slice(fb * NFB, (fb + 1) * NFB)
                    h_in_chunks = [hT_all[:, kc, fsl] for kc in range(HC)]
                    chains.append(dict(fb=fb, w="g", ws=r1_ws, bs=r1_bs,
                                       ins=h_in_chunks
                                       + [hT0_sb[:, gsl, 0:NR]]))
                    chains.append(dict(fb=fb, w="v", ws=r2_ws, bs=r2_bs,
                                       ins=h_in_chunks))
                relueng = [None, nc.vector, None, nc.vector]

                def relu_from(dst, src, bias, ci, eng="auto"):
                    if eng == "split":
                        # half-width on Act + DVE in parallel: ~350ns vs
                        # ~600ns for the full tile (tail is relu-latency
                        # bound)
                        hw_ = NFB // 2
                        nc.scalar.activation(dst[:, 0:hw_], src[:, 0:hw_],
                                             AF.Relu, bias=bias)
                        nc.vector.tensor_scalar(dst[:, hw_:], src[:, hw_:],
                                                bias, 0.0,
                                                op0=ALU.add, op1=ALU.max)
                        return
                    if eng == "auto":
                        eng = relueng[ci]
                    if eng is None:
                        nc.scalar.activation(dst, src, AF.Relu, bias=bias)
                    else:
                        eng.tensor_scalar(dst, src, bias, 0.0,
                                          op0=ALU.add, op1=ALU.max)

                def ro_l0(ch, ci, eng="auto"):
                    key = f"{ch['w']}{ch['fb']}"
                    p = ps.tile([128, NFB], f32,
                                tag="pP" if ci % 2 == 0 else "pG2", bufs=2,
                                name=f"rop0_{key}")
                    for i, (wap, rhs) in enumerate(zip(ch["ws"][0], ch["ins"])):
                        nc.tensor.matmul(p[:], wap, rhs, start=(i == 0),
                                         stop=(i == len(ch["ins"]) - 1))
                    a1 = wk.tile([128, NFB], f32r, tag="P", bufs=8,
                                 name=f"roa1_{key}")
                    relu_from(a1[:], p[:], ch["bs"][0], ci, eng)
                    ch["a1"] = a1

                def ro_l1(fb, eng="auto"):
                    for ci0, ch in enumerate(chains[2 * fb:2 * fb + 2]):
                        ci = 2 * fb + ci0
                        key = f"{ch['w']}{ch['fb']}"
                        ch["a2"] = []
                        for oc in range(2):
                            p2 = ps.tile([128, NFB], f32,
                                         tag="pP" if oc == 0 else "pG2",
                                         bufs=2, name=f"rop1_{key}_{oc}")
                            nc.tensor.matmul(p2[:], ch["ws"][1][oc],
                                             ch["a1"][:],
                                             start=True, stop=True)
                            t = wk.tile([128, NFB], f32r, tag="P", bufs=8,
                                        name=f"roa2_{key}_{oc}")
                            relu_from(t[:], p2[:], ch["bs"][1][oc],
                                      (ci + oc) % 2, eng)
                            ch["a2"].append(t)

                def ro_l2(fb, eng="auto"):
                    for ci0, ch in enumerate(chains[2 * fb:2 * fb + 2]):
                        key = f"{ch['w']}{ch['fb']}"
                        p3 = ps.tile([128, NFB], f32,
                                     tag="pP" if ci0 == 0 else "pG2",
                                     bufs=2, name=f"rop2_{key}")
                        for kc in range(2):
                            nc.tensor.matmul(p3[:], ch["ws"][2][kc],
                                             ch["a2"][kc][:],
                                             start=(kc == 0), stop=(kc == 1))
                        a3 = wk.tile([128, NFB], f32r, tag="P", bufs=8,
                                     name=f"roa3_{key}")
                        relu_from(a3[:], p3[:], ch["bs"][2], 0, eng)
                        ch["a3"] = a3

                def ro_l3(fb):
                    for ch in chains[2 * fb:2 * fb + 2]:
                        key = f"{ch['w']}{ch['fb']}"
                        p4 = ps.tile([TARGET, NFB], f32, tag="pGN", bufs=2,
                                     name=f"rop3_{key}")
                        nc.tensor.matmul(p4[:], ch["ws"][3], ch["a3"][:],
                                         start=True, stop=True)
                        ch["p4"] = p4

                def ro_finals(fb):
                    # finals: sum_v gate*val*mask per graph
                    fsl = slice(fb * NFB, (fb + 1) * NFB)
                    chg, chv = chains[2 * fb:2 * fb + 2]
                    vm = wk.tile([TARGET, NFB], f32, tag="z", bufs=2,
                                 name=f"vm_{fb}")
                    nc.vector.scalar_tensor_tensor(
                        vm[:], chv["p4"][:], r2b3_sb[:], mb_sb[:, fsl],
                        op0=ALU.add, op1=ALU.mult)
                    gate_s = wk.tile([TARGET, NFB], f32, tag="r", bufs=2,
                                     name=f"gate_{fb}")
                    nc.scalar.activation(gate_s[:], chg["p4"][:], AF.Sigmoid,
                                         bias=r1b3_sb[:])
                    for gg in range(GPB):
                        ga = fb * GPB + gg
                        sc = wk.tile([TARGET, NR], f32, tag="t1", bufs=2,
                                     name=f"sc_{fb}_{gg}")
                        nc.vector.scalar_tensor_tensor(
                            sc[:], gate_s[:, gg * NR:(gg + 1) * NR], 1.0,
                            vm[:, gg * NR:(gg + 1) * NR],
                            op0=ALU.mult, op1=ALU.mult,
                            accum_out=out_sb[:, ga:ga + 1])

                # ---- message passing steps ----
                for s in range(NSTEP):
                    hcs = [0] if s == 0 else list(range(HC))

                    # -- phase A (per graph): projections + agg -> mT_all --
                    def proj_lhsT(g, hc):
                        if s == 0:
                            assert hc == 0
                            if GPB <= g < GPB + 2:
                                return boot_hT0[g - GPB]
                            return hT0_sb[:, g, :]
                        return hT_all[:, hc, g * NR:(g + 1) * NR]

                    def projections(g):
                        P_sb = []
                        cpeng = [nc.vector.tensor_copy, nc.scalar.copy,
                                 nc.scalar.copy, nc.scalar.copy]
                        if s == 0:
                            cpeng[2] = nc.vector.tensor_copy
                        for l in range(L):
                            # s0: spread projection PSUM across pP + the
                            # (GRU-idle) pG2 pool
                            ptag = "pP" if l % 2 == 0 else "pG2"
                            pp = ps.tile([128, MSG], f32, tag=ptag, bufs=2,
                                         name=f"pp_{s}_{g}_{l}")
                            pv = pp[:] if s == 0 else pp[0:NR, :]
                            if s == 0 and GPB <= g < GPB + 2 and l == 0:
                                nc.tensor.matmul(pv, boot_hT0[g - GPB],
                                                 boot_A0,
                                                 start=True, stop=True)
                            else:
                                for i, hc in enumerate(hcs):
                                    nc.tensor.matmul(pv, proj_lhsT(g, hc),
                                                     A_sb[:, l, hc, :],
                                                     start=(i == 0),
                                                     stop=(i == len(hcs) - 1))
                            psb = wk.tile([128, MSG], f32r, tag="P", bufs=8,
                                          name=f"psb_{s}_{g}_{l}")
                            cpeng[l](psb[:], pp[:])
                            P_sb.append(psb)
                        return P_sb

                    def agg_m(g, P_sb):
                        mp = ps.tile([128, MSG], f32, tag="pMT", bufs=2,
                                     name=f"mp_{s}_{g}")
                        for l in range(L):
                            nc.tensor.matmul(mp[0:NR, :],
                                             mask_sb[:, g, l, 0:NR],
                                             P_sb[l][:],
                                             start=(l == 0), stop=(l == L - 1))
                        m_sb = wk.tile([128, MSG], f16, tag="m", bufs=4,
                                       name=f"m_{s}_{g}")
                        nc.vector.tensor_copy(m_sb[0:NR, :], mp[0:NR, :])
                        return m_sb

                    def mT_make(g, m_sb):
                        # s0: pGN is free until the D-phase; avoids 3-deep
                        # pMT pressure from the delayed-transpose pipeline
                        tp = ps.tile([128, MC, NR], f16,
                                     tag="pGN" if s == 0 else "pMT", bufs=2,
                                     name=f"tp_{s}_{g}")
                        for c in range(MC):
                            nc.tensor.transpose(
                                tp[:, c, :],
                                m_sb[0:NR, c * 128:(c + 1) * 128],
                                ident16_t[0:NR, 0:NR])
                        nc.scalar.copy(mT_all[:, :, g * NR:(g + 1) * NR],
                                       tp[:])

                    # fb1's graphs first every step: D(fb1) then needs only
                    # mid-A mT's, and E(s, fb0) consistently trails into the
                    # next A-phase's fb1 half (no fb-parity switch bubble)
                    gorder = list(range(GPB, G)) + list(range(GPB))
                    if s == 0:
                        # short s0 projections expose the agg->copy->
                        # transpose latency: delay each pair's transposes
                        # until after the next pair's projections
                        pending_m = []
                        for gp in range(G // 2):
                            g0, g1 = gorder[2 * gp], gorder[2 * gp + 1]
                            Ps0 = projections(g0)
                            Ps1 = projections(g1)
                            for g_, m_ in pending_m:
                                mT_make(g_, m_)
                            m0 = agg_m(g0, Ps0)
                            m1 = agg_m(g1, Ps1)
                            pending_m = [(g0, m0), (g1, m1)]
                        for g_, m_ in pending_m:
                            mT_make(g_, m_)
                    else:
                        for gp in range(G // 2):
                            g0, g1 = gorder[2 * gp], gorder[2 * gp + 1]
                            Ps0 = projections(g0)
                            Ps1 = projections(g1)
                            mT_make(g0, agg_m(g0, Ps0))
                            mT_make(g1, agg_m(g1, Ps1))

                    # -- phase D+E (per fb, per h-chunk c): gate GEMMs + GRU --
                    def emit_mms(o, mms):
                        for i, (lh, rh, pm) in enumerate(mms):
                            nc.tensor.matmul(o, lh, rh, start=(i == 0),
                                             stop=(i == len(mms) - 1),
                                             perf_mode=pm)

                    fborder = [1, 0]
                    for fb in fborder:
                        fbsl = slice(fb * NFB, (fb + 1) * NFB)
                        for c in range(HC):
                            # gate chunks: r=c, z=4+c, n(i)=8+c, n(h)=8+c
                            r_ps = ps.tile([128, NFB], f32, tag="pP", bufs=2,
                                           name=f"rps_{s}_{fb}_{c}")
                            z_ps = ps.tile([128, NFB], f32, tag="pG2", bufs=2,
                                           name=f"zps_{s}_{fb}_{c}")
                            gin_ps = ps.tile([128, NFB], f32, tag="pGN",
                                             bufs=2, name=f"gin_{s}_{fb}_{c}")
                            ghn_ps = ps.tile([128, NFB], f32, tag="pMT",
                                             bufs=2, name=f"ghn_{s}_{fb}_{c}")
                            # ghn: gh chunks (+ bias plane) only, no wih
                            ghn_mms = []
                            if s == 0:
                                ghn_mms.append((
                                    s0ghn8_t[:, :, c * 128:(c + 1) * 128],
                                    hT08[:, :, fbsl], DR))
                            else:
                                for c2 in (0, 2):
                                    ghn_mms.append((
                                        whh8T_sb[:, c2:c2 + 2,
                                                 (8 + c) * 128:(9 + c) * 128],
                                        hT8_all[:, s % 2, c2:c2 + 2, fbsl],
                                        DR))
                            emit_mms(ghn_ps[:], ghn_mms)

                            def rz_mms(gc):
                                csl = slice(gc * 128, (gc + 1) * 128)
                                mms = []
                                if s == 0:
                                    mms.append((s0rz8_t[:, :, csl],
                                                hT08[:, :, fbsl], DR))
                                else:
                                    for c2 in (0, 2):
                                        mms.append((
                                            whh8T_sb[:, c2:c2 + 2, csl],
                                            hT8_all[:, s % 2, c2:c2 + 2,
                                                    fbsl], DR))
                                for c2 in range(MC):
                                    mms.append((wihT_sb[:, c2, csl],
                                                mT_all[:, c2, fbsl], None))
                                return mms

                            emit_mms(r_ps[:], rz_mms(c))
                            emit_mms(z_ps[:], rz_mms(4 + c))
                            gin_mms = [(wihT_sb[:, c2,
                                                (8 + c) * 128:(9 + c) * 128],
                                        mT_all[:, c2, fbsl],
                                        None) for c2 in range(MC)]
                            emit_mms(gin_ps[:], gin_mms)

                            # -- E: gate nonlinearities + state update --
                            r_sb = wk.tile([128, NFB], f32, tag="r", bufs=2,
                                           name=f"r_{s}_{fb}_{c}")
                            nc.scalar.activation(r_sb[:], r_ps[:], AF.Sigmoid,
                                                 scale=1.0 / GSC,
                                                 bias=gbias_t[:, c:c + 1])
                            if not (s == 0 and c > 0):
                                z_sb = wk.tile([128, NFB], f32, tag="z",
                                               bufs=2, name=f"z_{s}_{fb}_{c}")
                                nc.scalar.activation(
                                    z_sb[:], z_ps[:], AF.Sigmoid,
                                    scale=1.0 / GSC,
                                    bias=gbias_t[:, 4 + c:5 + c])
                            u_sb = wk.tile([128, NFB], f32, tag="u", bufs=2,
                                           name=f"u_{s}_{fb}_{c}")
                            nc.scalar.activation(
                                u_sb[:], z_ps[:], AF.Sigmoid,
                                scale=-1.0 / GSC,
                                bias=gbias_t[:, 8 + c:9 + c])
                            # rhn16 = (ghn + 16*bhn)*r = 16*r*h_n; npre =
                            # gin + rhn16 = 16*(i_n - bin + r*h_n); the 1/16
                            # folds into tanh's scale, bin into its bias
                            rhn = wk.tile([128, NFB], f32, tag="t1", bufs=2,
                                          name=f"rhn_{s}_{fb}_{c}")
                            nc.vector.scalar_tensor_tensor(
                                rhn[:], ghn_ps[:],
                                gbias_t[:, 16 + c:17 + c], r_sb[:],
                                op0=ALU.add, op1=ALU.mult)
                            npre = wk.tile([128, NFB], f32, tag="t2", bufs=2,
                                           name=f"npre_{s}_{fb}_{c}")
                            nc.vector.tensor_add(npre[:], gin_ps[:], rhn[:])
                            n_sb = wk.tile([128, NFB], f32, tag="n", bufs=2,
                                           name=f"n_{s}_{fb}_{c}")
                            nc.scalar.activation(n_sb[:], npre[:], AF.Tanh,
                                                 scale=1.0 / GSC,
                                                 bias=gbias_t[:, 12 + c:13 + c])
                            # h' = (1-z)*n + z*h. zh is off the critical path
                            # (ready before n) -> Pool; un/h'add gate the
                            # next step's proj lhsT -> keep on DVE
                            hsl = hT_all[:, c, fbsl]
                            if s == 0:
                                if c == 0:
                                    zh = wk.tile([128, NFB], f32, tag="zh",
                                                 bufs=2, name=f"zh_{s}_{fb}")
                                    nc.gpsimd.tensor_mul(
                                        zh[:], z_sb[:],
                                        hT0_sb[:, fb * GPB:(fb + 1) * GPB,
                                               0:NR])
                                    un = wk.tile([128, NFB], f32, tag="un",
                                                 bufs=2, name=f"un_{s}_{fb}")
                                    nc.gpsimd.tensor_mul(un[:], u_sb[:],
                                                         n_sb[:])
                                    nc.gpsimd.tensor_add(hsl, un[:], zh[:])
                                else:
                                    nc.gpsimd.tensor_mul(hsl, u_sb[:],
                                                         n_sb[:])
                            else:
                                zh = wk.tile([128, NFB], f32, tag="zh",
                                             bufs=2, name=f"zh_{s}_{fb}_{c}")
                                nc.gpsimd.tensor_mul(zh[:], z_sb[:], hsl)
                                un = wk.tile([128, NFB], f32, tag="un",
                                             bufs=2, name=f"un_{s}_{fb}_{c}")
                                eng_un = (nc.gpsimd if s == NSTEP - 1
                                          else nc.vector)
                                eng_un.tensor_mul(un[:], u_sb[:], n_sb[:])
                                eng_un.tensor_add(hsl, un[:], zh[:])
                            if s < NSTEP - 1:
                                nc.gpsimd.tensor_scalar_mul(
                                    hT8_all[:, (s + 1) % 2, c, fbsl],
                                    hsl, 1.0)
                            if s == NSTEP - 1 and fb == fborder[-1] and c >= 2:
                                # fb1's hT is done (its D/E ran first): start
                                # its readout L0 between fb0's D chunks so
                                # the relus queue ahead of fb0's E tail
                                ro_l0(chains[2 + (c - 2)], 2 + (c - 2))

                        if s == NSTEP - 1 and fb == fborder[-1]:
                            # fb1's hT is complete (its D/E ran first):
                            # overlap its readout L0..L2 with fb0's E tail
                            ro_l1(1)
                            ro_l2(1)

                # zippered readout, fb1 first
                ro_l0(chains[0], 0)
                ro_l0(chains[1], 1)
                ro_l1(0)
                ro_l3(1)
                ro_l2(0)
                ro_finals(1)
                ro_l3(0)
                ro_finals(0)
                nc.sync.dma_start(d_out[:], out_sb[:])

    nc.compile()
    return nc


def _prep_core_inputs(core, g_, h_in, e):
    cs = slice(core * G, (core + 1) * G)
    f = np.float32
    hT0 = np.ascontiguousarray(h_in[cs].transpose(2, 0, 1))  # [F, G, N]
    labels = np.arange(1, L + 1, dtype=f)
    # mask[w, g, l, v] = (e[g, v, w] == l+1)
    e_c = e[cs]  # [G, V, W]
    oh = (e_c[:, None, :, :] == labels[None, :, None, None]).astype(f)
    mask = np.ascontiguousarray(oh.transpose(3, 0, 1, 2))  # [W, G, L, V]
    if MASK8_DMA:
        import ml_dtypes
        mask = mask.astype(ml_dtypes.float8_e4m3)
    # readout mask broadcast (reference: sum(h0, -1) != 0), packed to 112
    rmask = (h_in[cs].sum(-1) != 0).astype(f)[:, 0:NR]      # [G, NR]
    mb = np.broadcast_to(rmask.reshape(1, G * NR),
                         (TARGET, G * NR)).copy()
    return {
        "hT0": hT0,
        "mask": mask,
        "mb": mb,
    }


def _prep_shared_inputs(A, gru_Wih, gru_Whh, gru_bih, gru_bhh,
                        r1_Ws, r1_bs, r2_Ws, r2_bs):
    f = np.float32

    def chunk_rows(M, nch):  # [K, C] -> [128, nch, C] with K = nch*128
        K, C = M.shape
        assert K == nch * 128
        return np.ascontiguousarray(M.reshape(nch, 128, C).transpose(1, 0, 2))

    import ml_dtypes
    f8np = ml_dtypes.float8_e4m3
    GSCf = np.float32(GSC)

    A_t = np.ascontiguousarray(
        A.reshape(L, HC, 128, MSG).transpose(2, 0, 1, 3))  # [128, L, HC, MSG]

    # transposed GRU weights: [feat, chunk, gate-col], premultiplied by GSC
    def t_weights(W, nch):  # W [3H, K] -> [128(f), nch, GC*128]
        Wt = (W * GSCf).reshape(GC, 128, nch, 128)  # [gc, j, kc, f]
        return np.ascontiguousarray(
            Wt.transpose(3, 2, 0, 1).reshape(128, nch, GC * 128))

    wihT = t_weights(np.asarray(gru_Wih, f), MC)
    whh8T_f = t_weights(np.asarray(gru_Whh, f), HC)
    whh8T = whh8T_f.astype(f8np)

    bih = np.asarray(gru_bih, f)
    bhh = np.asarray(gru_bhh, f)
    brz = (bih + bhh)[:2 * H]
    bin_ = bih[2 * H:]
    bhn = bhh[2 * H:]
    # gbias columns: r(0:4) z(4:8) -z(8:12) bin(12:16) bhn*GSC(16:20)
    gbias = np.zeros((128, 20), f)
    for c in range(4):
        gbias[:, c] = brz[c * 128:(c + 1) * 128]
        gbias[:, 4 + c] = brz[H + c * 128:H + (c + 1) * 128]
        gbias[:, 8 + c] = -brz[H + c * 128:H + (c + 1) * 128]
        gbias[:, 12 + c] = bin_[c * 128:(c + 1) * 128]
        gbias[:, 16 + c] = GSCf * bhn[c * 128:(c + 1) * 128]
    # step-0 lhsT pairs
    s0rz8 = np.zeros((128, 2, 2 * H), f)
    s0rz8[:, 0, :] = whh8T_f[:, 0, 0:2 * H]
    s0ghn8 = np.zeros((128, 2, H), f)
    s0ghn8[:, 0, :] = whh8T_f[:, 0, 2 * H:3 * H]

    # readout weights, transposed layout
    r1w0t = np.ascontiguousarray(r1_Ws[0].T)  # [2H, 128]
    r1w0 = np.zeros((128, 5, 128), f)
    for kc in range(4):
        r1w0[:, kc, :] = r1w0t[kc * 128:(kc + 1) * 128]
    r1w0[:, 4, :] = r1w0t[H:H + F_IN]  # h0 chunk (features 0:128 of h0 half)
    r1w1 = np.ascontiguousarray(r1_Ws[1].T.reshape(128, 2, 128))
    r1w2 = chunk_rows(np.ascontiguousarray(r1_Ws[2].T), 2)
    r1w3 = np.ascontiguousarray(r1_Ws[3].T)  # [128, 12]
    r2w0 = chunk_rows(np.ascontiguousarray(r2_Ws[0].T), 4)
    r2w1 = np.ascontiguousarray(r2_Ws[1].T.reshape(128, 2, 128))
    r2w2 = chunk_rows(np.ascontiguousarray(r2_Ws[2].T), 2)
    r2w3 = np.ascontiguousarray(r2_Ws[3].T)

    identcol = np.concatenate([np.eye(128, dtype=f), np.ones((128, 1), f)], 1)
    rowb = np.concatenate([r1w0, r1w1, r1w2, r2w0, r2w1, r2w2], axis=1)
    row3 = np.stack([r1w3, r2w3], axis=1)
    robias = np.concatenate([
        r1_bs[0].reshape(-1, 1).astype(f),
        np.ascontiguousarray(r1_bs[1].reshape(2, 128).T),
        r1_bs[2].reshape(-1, 1).astype(f),
        r2_bs[0].reshape(-1, 1).astype(f),
        np.ascontiguousarray(r2_bs[1].reshape(2, 128).T),
        r2_bs[2].reshape(-1, 1).astype(f)], axis=1)
    rob12 = np.concatenate([r1_bs[3].reshape(-1, 1).astype(f),
                            r2_bs[3].reshape(-1, 1).astype(f)], axis=1)
    return {
        "A": A_t,
        "wihT": wihT,
        "whh8T": np.ascontiguousarray(whh8T),
        "s0rz8": s0rz8.astype(f8np),
        "s0ghn8": s0ghn8.astype(f8np),
        "gbias": gbias,
        "identcol": np.ascontiguousarray(identcol),
        "rowb": np.ascontiguousarray(rowb),
        "row3": np.ascontiguousarray(row3),
        "robias": np.ascontiguousarray(robias),
        "rob12": np.ascontiguousarray(rob12),
    }


def _get_nc(nreps=1):
    key = ("nc", nreps)
    if key not in _CACHE:
        _CACHE[key] = _build(nreps)
    return _CACHE[key]


def _run(in_maps, **kwargs):
    nc = _get_nc()
    return run_bass_kernel_spmd(nc, in_maps, core_ids=list(range(NCORES)),
                                **kwargs)


def make_in_maps(g, h_in, e, A, gru_Wih, gru_Whh, gru_bih, gru_bhh,
                 r1_W0, r1_b0, r1_W1, r1_b1, r1_W2, r1_b2, r1_W3, r1_b3,
                 r2_W0, r2_b0, r2_W1, r2_b1, r2_W2, r2_b2, r2_W3, r2_b3):
    r1_Ws, r1_bs = [r1_W0, r1_W1, r1_W2, r1_W3], [r1_b0, r1_b1, r1_b2, r1_b3]
    r2_Ws, r2_bs = [r2_W0, r2_W1, r2_W2, r2_W3], [r2_b0, r2_b1, r2_b2, r2_b3]
    arrs = {k: np.asarray(v, np.float32) for k, v in dict(
        g=g, h_in=h_in, e=e, A=A, gru_Wih=gru_Wih, gru_Whh=gru_Whh,
        gru_bih=gru_bih, gru_bhh=gru_bhh).items()}
    r1_Ws = [np.asarray(w, np.float32) for w in r1_Ws]
    r1_bs = [np.asarray(b, np.float32) for b in r1_bs]
    r2_Ws = [np.asarray(w, np.float32) for w in r2_Ws]
    r2_bs = [np.asarray(b, np.float32) for b in r2_bs]
    shared = _prep_shared_inputs(arrs["A"], arrs["gru_Wih"], arrs["gru_Whh"],
                                 arrs["gru_bih"], arrs["gru_bhh"],
                                 r1_Ws, r1_bs, r2_Ws, r2_bs)
    f = np.float32
    in_maps = []
    for core in range(NCORES):
        m = dict(shared)
        m.update(_prep_core_inputs(core, arrs["g"], arrs["h_in"], arrs["e"]))
        boot = np.concatenate([m["hT0"][:, GPB, :], m["hT0"][:, GPB + 1, :],
                               np.asarray(arrs["A"][0, 0:128, :], f)], 1)
        m["boot"] = np.ascontiguousarray(boot)
        in_maps.append(m)
    return in_maps


def kernel(**inputs):
    in_maps = make_in_maps(**inputs)
    res = _run(in_maps)
    out = np.zeros((B, TARGET), np.float32)
    for core in range(NCORES):
        out[core * G:(core + 1) * G] = res.results[core]["out"].T
    return out


if __name__ == "__main__":
    import reference
    inputs = {k: np.asarray(v) for k, v in reference.setup_inputs().items()}
    expected = np.asarray(reference.reference(**inputs))
    actual = kernel(**inputs)
    scale = np.abs(expected).max()
    err = np.abs(actual - expected).max() / scale
    print("Relative error:", err)


# revision 89
# speedup vs baseline: 1.0122x; 1.0012x over previous
"""MPNN-GGNN forward on 8 Trainium2 NeuronCores.

Data-parallel over the batch: 8 graphs per core. All weights replicated.
Per-core Bass/Tile kernel computes 4 message-passing + GRU steps and the
gated readout entirely on-chip; f32r matmuls at full PE rate, gh in fp8e4
DoubleRow.

v2 (281us -> 233us): the GRU runs in TRANSPOSED (feature-major) layout
with the node dimension PACKED to the 112 real nodes (the reference's
node_on = arange(128) < 112 is structural). Gates are computed as
out[gate_chunk(128), fb*node(448)] with GRU weights as stationary lhsT
([feat, gate] chunks) and mT / hT8 as the moving rhs spanning one
readout free-block (4 graphs x 112 nodes) per instruction. Wins vs v1:
  - GRU biases become per-partition columns, all exact f32 and all
    free: r/z/u/bin via the Act engine's func(x*scale + bias), bhn via
    the rhn STT's scalar-AP slot ((ghn + 16*bhn)*r, with the 1/16
    rescale folded into tanh's scale). Zero bias matmuls remain.
  - h' emerges feature-major = exactly what the next step's projection
    lhsT and the readout rhs need: all 32 h-transposes and their
    PSUM->SBUF copies disappear.
  - packed-112 free dims cut every gate GEMM / GRU elementwise /
    m-transpose / readout matmul by 12.5%.
  - m is rounded to fp16 before its PE transpose (1.0 vs 1.5 cyc/row;
    fp16's 2^-11 step measured 0.006 standalone err vs bf16's fatal
    0.029); mT upconverts back to f32r exactly in the PSUM->SBUF copy.
  - hT8 is parity-double-buffered (D(s) reads s%2, E(s) writes (s+1)%2)
    since the DR chunk-pairs span all 4 h-chunks.
  - uniform fb1-first schedule: A-phase graph order [4..7,0..3] and
    D/E fb-order [1,0] every step, so D(fb1) only needs mid-A mT's and
    the trailing E(fb0) chain is always covered by the next A-phase
    (no fb-parity-switch bubble); boot DMA carries hT0 for g4/g5.
  - GRU elementwise spread: Act 4 activations, DVE the 2 PSUM-reading
    ops, Pool (idle otherwise) zh/un/h' + the fp8 cast.
  - s3: fb1's readout L0 is injected between fb0's gate-GEMM chunk
    groups; readout layers zipper fb1/fb0 as before.

Layout conventions per core (G = 8 graphs, NR = 112 nodes, H = 512):
  hT_all [128(feat), HC, G*NR] f32r  feature-major hidden state
  hT8_all[128, 2, HC, G*NR] f8       fp8 copy for DoubleRow gh
  mT_all [128, MC, G*NR] f32r        message^T, rhs of the gate GEMMs
  mask_sb[128(w), G, L, 128(v)]      (e^T == l+1) one-hot adjacency
  matmul convention: out[i,j] = sum_k lhsT[k,i] * rhs[k,j]
"""

import numpy as np

import concourse.mybir as mybir
import concourse.tile as tile
from concourse import bacc
from concourse.bass_utils import run_bass_kernel_spmd

# problem constants (hardcoded per contract)
B, N, F_IN = 64, 128, 128
H, MSG, L = 512, 512, 4
NSTEP = 4
TARGET = 12
NCORES = 8
G = B // NCORES          # graphs per core
HC = H // 128            # h chunks
MC = MSG // 128          # msg chunks
GC = 3 * H // 128        # gate chunks (12)
FB = 2                   # readout free blocks (4 graphs x 112 nodes each)
GPB = G // FB
NR = 112                 # real nodes per graph (reference: arange(N) < 112)
NFB = GPB * NR           # packed free size per fb (448)

f32 = mybir.dt.float32
f32r = mybir.dt.float32r
f16 = mybir.dt.float16
f8 = mybir.dt.float8e4
AF = mybir.ActivationFunctionType
ALU = mybir.AluOpType
AX = mybir.AxisListType
DR = mybir.MatmulPerfMode.DoubleRow
GSC = 16.0  # gate-preact PSUM scale: wihT/whh8T premultiplied by 16
MASK8_DMA = True

_CACHE = {}


def _build(nreps=1):
    nc = bacc.Bacc("TRN2", target_bir_lowering=False)

    # ---- DRAM I/O ----
    # boot: hT0 for graphs 0-1 + A[l=0, hc=0] packed in one early DMA
    d_boot = nc.dram_tensor("boot", [128, 2 * N + MSG], f32r,
                            kind="ExternalInput")
    d_hT0 = nc.dram_tensor("hT0", [F_IN, G, N], f32r, kind="ExternalInput")
    d_mask = nc.dram_tensor("mask", [N, G, L, N],
                            f8 if MASK8_DMA else f32r, kind="ExternalInput")
    d_A = nc.dram_tensor("A", [128, L, HC, MSG], f32r, kind="ExternalInput")
    # transposed GRU weights: [feat, chunk, gate-col]
    d_wihT = nc.dram_tensor("wihT", [128, MC, GC * 128], f32r,
                            kind="ExternalInput")
    d_whh8T = nc.dram_tensor("whh8T", [128, HC, GC * 128], f8,
                             kind="ExternalInput")
    # step-0 lhsT pairs: rz (whh chunk0, 0), ghn (whh_n chunk0, 0)
    d_s0rz8 = nc.dram_tensor("s0rz8", [128, 2, 2 * H], f8,
                             kind="ExternalInput")
    d_s0ghn8 = nc.dram_tensor("s0ghn8", [128, 2, H], f8, kind="ExternalInput")
    # gate bias columns (f32): r(0:4) z(4:8) -z(8:12) bin(12:16)
    # bhn*GSC(16:20) -- the ghn bias rides in the rhn STT, not a matmul
    d_gbias = nc.dram_tensor("gbias", [128, 20], f32, kind="ExternalInput")
    # readout mask broadcast, computed host-side from h_in (sum(h0) != 0)
    d_mb = nc.dram_tensor("mb", [TARGET, G * NR], f32, kind="ExternalInput")
    d_identcol = nc.dram_tensor("identcol", [128, 129], f32r,
                                kind="ExternalInput")
    d_rowb = nc.dram_tensor("rowb", [128, 17, 128], f32r, kind="ExternalInput")
    d_row3 = nc.dram_tensor("row3", [128, 2, TARGET], f32r,
                            kind="ExternalInput")
    d_robias = nc.dram_tensor("robias", [128, 8], f32, kind="ExternalInput")
    d_rob12 = nc.dram_tensor("rob12", [TARGET, 2], f32, kind="ExternalInput")
    d_out = nc.dram_tensor("out", [TARGET, G], f32, kind="ExternalOutput")

    with tile.TileContext(nc) as tc:
        with tc.tile_pool(name="st", bufs=1) as st, \
             tc.tile_pool(name="state", bufs=1) as stt, \
             tc.tile_pool(name="wk", bufs=2) as wk, \
             tc.tile_pool(name="ps", bufs=1, space="PSUM") as ps:

            # ---- static loads, in consumption order ----
            boot_t = st.tile([128, 2 * N + MSG], f32r, tag="boot")
            nc.sync.dma_start(boot_t[:], d_boot[:])
            # p-state prewarm: the PE ramps 0.65->1.2->2.4 GHz over ~3us of
            # continuous busy. Fill the boot-DMA wait with zero matmuls on a
            # memset tile so real work starts at full clock.
            zwarm = st.tile([128, 256], f32r, tag="zwarm")
            nc.gpsimd.memset(zwarm[:].bitcast(f32), 0.0)
            pwarm = ps.tile([128, 256], f32, tag="pP", bufs=2, name="pwarm")
            for wi in range(12):
                nc.tensor.matmul(pwarm[:], zwarm[:, 0:128], zwarm[:],
                                 start=(wi == 0), stop=(wi == 11))
            boot_hT0 = [boot_t[:, 0:N], boot_t[:, N:2 * N]]
            boot_A0 = boot_t[:, 2 * N:2 * N + MSG]
            hT0_sb = st.tile([F_IN, G, N], f32r, tag="hT0")
            A_sb = st.tile([128, L, HC, MSG], f32r, tag="A")
            nc.sync.dma_start(A_sb[:, 1, 0, :], d_A[:, 1, 0, :])
            nc.sync.dma_start(A_sb[:, 2, 0, :], d_A[:, 2, 0, :])
            nc.sync.dma_start(A_sb[:, 3, 0, :], d_A[:, 3, 0, :])
            identcol_t = st.tile([128, 129], f32r, tag="identcol")
            nc.sync.dma_start(identcol_t[:], d_identcol[:])
            ident_sb = identcol_t[:, 0:128]
            onescol_sb = identcol_t[:, 128:129]
            # fp16 identity: m transposes run at 1.0 cyc/row (vs 1.5 f32r);
            # m is rounded to fp16 (2^-11, measured 0.006 standalone err)
            ident16_t = st.tile([128, 128], f16, tag="ident16")
            nc.gpsimd.tensor_scalar_mul(ident16_t[:], ident_sb, 1.0)
            mask_sb = st.tile([N, G, L, N], f32r, tag="mask")
            mask8st = None
            if MASK8_DMA:
                mask8st = st.tile([N, 2, L, N], f8, tag="mask8st")

            def mask_load(g_):
                if MASK8_DMA:
                    sl8 = g_ % 2
                    nc.sync.dma_start(mask8st[:, sl8, :, :],
                                      d_mask[:, g_, :, :])
                    nc.gpsimd.tensor_scalar_mul(mask_sb[:, g_, :, :],
                                                mask8st[:, sl8, :, :], 1.0)
                else:
                    nc.sync.dma_start(mask_sb[:, g_, :, :],
                                      d_mask[:, g_, :, :])

            mask_load(GPB)
            mask_load(GPB + 1)
            nc.sync.dma_start(hT0_sb[:], d_hT0[:])
            nc.sync.dma_start(A_sb[:, 0, 0, :], d_A[:, 0, 0, :])
            for g_ in [GPB + 2, GPB + 3] + list(range(GPB)):
                mask_load(g_)
            wihT_sb = st.tile([128, MC, GC * 128], f32r, tag="wihT")
            for c in range(MC):
                nc.sync.dma_start(wihT_sb[:, c, :], d_wihT[:, c, :])
            gbias_t = st.tile([128, 20], f32, tag="gbias")
            nc.sync.dma_start(gbias_t[:], d_gbias[:])
            s0rz8_t = st.tile([128, 2, 2 * H], f8, tag="s0rz8")
            nc.sync.dma_start(s0rz8_t[:], d_s0rz8[:])
            s0ghn8_t = st.tile([128, 2, H], f8, tag="s0ghn8")
            nc.sync.dma_start(s0ghn8_t[:], d_s0ghn8[:])

            # state tiles
            hT_all = stt.tile([128, HC, G * NR], f32r, tag="hT_all")
            # parity-double-buffered: D(s) reads parity s%2 while E(s)
            # writes parity (s+1)%2 (the DR pairs span all 4 chunks, so a
            # single buffer would RAW-hazard against the per-chunk updates)
            hT8_all = stt.tile([128, 2, HC, G * NR], f8, tag="hT8_all")
            hT08 = stt.tile([128, 2, G * NR], f8, tag="hT08")
            # chunk 1 = zero pair partner for s0's single-chunk DR groups
            nc.gpsimd.memset(hT08[:, 1, :], 0.0)
            for g_ in range(G):
                nc.gpsimd.tensor_scalar_mul(
                    hT08[:, 0, g_ * NR:(g_ + 1) * NR],
                    hT0_sb[:, g_, 0:NR], 1.0)
            mT_all = stt.tile([128, MC, G * NR], f32r, tag="mT_all")

            mb_sb = st.tile([TARGET, G * NR], f32, tag="mb_sb")
            nc.sync.dma_start(mb_sb[:], d_mb[:])
            for hc_ in range(1, HC):
                for l_ in range(L):
                    nc.sync.dma_start(A_sb[:, l_, hc_, :], d_A[:, l_, hc_, :])
            whh8T_sb = st.tile([128, HC, GC * 128], f8, tag="whh8T")
            nc.sync.dma_start(whh8T_sb[:], d_whh8T[:])

            rowb_t = st.tile([128, 17, 128], f32r, tag="rowb")
            nc.sync.dma_start(rowb_t[:], d_rowb[:])
            r1w0_sb = rowb_t[:, 0:5, :]
            r1w1_sb = rowb_t[:, 5:7, :]
            r1w2_sb = rowb_t[:, 7:9, :]
            r2w0_sb = rowb_t[:, 9:13, :]
            r2w1_sb = rowb_t[:, 13:15, :]
            r2w2_sb = rowb_t[:, 15:17, :]
            row3_t = st.tile([128, 2, TARGET], f32r, tag="row3")
            nc.sync.dma_start(row3_t[:], d_row3[:])
            r1w3_sb = row3_t[:, 0, :]
            r2w3_sb = row3_t[:, 1, :]
            robias_t = st.tile([128, 8], f32, tag="robias")
            nc.sync.dma_start(robias_t[:], d_robias[:])
            r1b0_sb = robias_t[:, 0:1]
            r1b1_sb = robias_t[:, 1:3]
            r1b2_sb = robias_t[:, 3:4]
            r2b0_sb = robias_t[:, 4:5]
            r2b1_sb = robias_t[:, 5:7]
            r2b2_sb = robias_t[:, 7:8]
            rob12_t = st.tile([TARGET, 2], f32, tag="rob12")
            nc.sync.dma_start(rob12_t[:], d_rob12[:])
            r1b3_sb = rob12_t[:, 0:1]
            r2b3_sb = rob12_t[:, 1:2]

            for _rep in range(nreps):
                # NOTE: the reference's per-step node_mask multiply is
                # dropped: masked inputs guarantee no edges touch virtual
                # nodes, their per-node GRU lanes never mix into real nodes,
                # and the readout re-applies mask_row.

                # ---- readout (layer-major over 4 independent chains) ----
                out_sb = st.tile([TARGET, G], f32, tag="out_sb")
                r1_ws = [[r1w0_sb[:, kc, :] for kc in range(5)],
                         [r1w1_sb[:, oc, :] for oc in range(2)],
                         [r1w2_sb[:, kc, :] for kc in range(2)],
                         r1w3_sb[:]]
                r1_bs = [r1b0_sb[:],
                         [r1b1_sb[:, oc:oc + 1] for oc in range(2)],
                         r1b2_sb[:]]
                r2_ws = [[r2w0_sb[:, kc, :] for kc in range(4)],
                         [r2w1_sb[:, oc, :] for oc in range(2)],
                         [r2w2_sb[:, kc, :] for kc in range(2)],
                         r2w3_sb[:]]
                r2_bs = [r2b0_sb[:],
                         [r2b1_sb[:, oc:oc + 1] for oc in range(2)],
                         r2b2_sb[:]]
                chains = []
                for fb in range(FB):
                    gsl = slice(fb * GPB, (fb + 1) * GPB)
                    fsl = slice(fb * NFB, (fb + 1) * NFB)
                    h_in_chunks = [hT_all[:, kc, fsl] for kc in range(HC)]
                    chains.append(dict(fb=fb, w="g", ws=r1_ws, bs=r1_bs,
                                       ins=h_in_chunks
                                       + [hT0_sb[:, gsl, 0:NR]]))
                    chains.append(dict(fb=fb, w="v", ws=r2_ws, bs=r2_bs,
                                       ins=h_in_chunks))
                relueng = [None, nc.vector, None, nc.vector]

                def relu_from(dst, src, bias, ci, eng="auto"):
                    if eng == "split":
                        # half-width on Act + DVE in parallel: ~350ns vs
                        # ~600ns for the full tile (tail is relu-latency
                        # bound)
                        hw_ = NFB // 2
                        nc.scalar.activation(dst[:, 0:hw_], src[:, 0:hw_],
                                             AF.Relu, bias=bias)
                        nc.vector.tensor_scalar(dst[:, hw_:], src[:, hw_:],
                                                bias, 0.0,
                                                op0=ALU.add, op1=ALU.max)
                        return
                    if eng == "auto":
                        eng = relueng[ci]
                    if eng is None:
                        nc.scalar.activation(dst, src, AF.Relu, bias=bias)
                    else:
                        eng.tensor_scalar(dst, src, bias, 0.0,
                                          op0=ALU.add, op1=ALU.max)

                def ro_l0(ch, ci, eng="auto"):
                    key = f"{ch['w']}{ch['fb']}"
                    p = ps.tile([128, NFB], f32,
                                tag="pP" if ci % 2 == 0 else "pG2", bufs=2,
                                name=f"rop0_{key}")
                    for i, (wap, rhs) in enumerate(zip(ch["ws"][0], ch["ins"])):
                        nc.tensor.matmul(p[:], wap, rhs, start=(i == 0),
                                         stop=(i == len(ch["ins"]) - 1))
                    a1 = wk.tile([128, NFB], f32r, tag="P", bufs=8,
                                 name=f"roa1_{key}")
                    relu_from(a1[:], p[:], ch["bs"][0], ci, eng)
                    ch["a1"] = a1

                def ro_l1(fb, eng="auto"):
                    for ci0, ch in enumerate(chains[2 * fb:2 * fb + 2]):
                        ci = 2 * fb + ci0
                        key = f"{ch['w']}{ch['fb']}"
                        ch["a2"] = []
                        for oc in range(2):
                            p2 = ps.tile([128, NFB], f32,
                                         tag="pP" if oc == 0 else "pG2",
                                         bufs=2, name=f"rop1_{key}_{oc}")
                            nc.tensor.matmul(p2[:], ch["ws"][1][oc],
                                             ch["a1"][:],
                                             start=True, stop=True)
                            t = wk.tile([128, NFB], f32r, tag="P", bufs=8,
                                        name=f"roa2_{key}_{oc}")
                            relu_from(t[:], p2[:], ch["bs"][1][oc],
                                      (ci + oc) % 2, eng)
                            ch["a2"].append(t)

                def ro_l2(fb, eng="auto"):
                    for ci0, ch in enumerate(chains[2 * fb:2 * fb + 2]):
                        key = f"{ch['w']}{ch['fb']}"
                        p3 = ps.tile([128, NFB], f32,
                                     tag="pP" if ci0 == 0 else "pG2",
                                     bufs=2, name=f"rop2_{key}")
                        for kc in range(2):
                            nc.tensor.matmul(p3[:], ch["ws"][2][kc],
                                             ch["a2"][kc][:],
                                             start=(kc == 0), stop=(kc == 1))
                        a3 = wk.tile([128, NFB], f32r, tag="P", bufs=8,
                                     name=f"roa3_{key}")
                        relu_from(a3[:], p3[:], ch["bs"][2], 0, eng)
                        ch["a3"] = a3

                def ro_l3(fb):
                    for ch in chains[2 * fb:2 * fb + 2]:
                        key = f"{ch['w']}{ch['fb']}"
                        p4 = ps.tile([TARGET, NFB], f32, tag="pGN", bufs=2,
                                     name=f"rop3_{key}")
                        nc.tensor.matmul(p4[:], ch["ws"][3], ch["a3"][:],
                                         start=True, stop=True)
                        ch["p4"] = p4

                def ro_finals(fb):
                    # finals: sum_v gate*val*mask per graph
                    fsl = slice(fb * NFB, (fb + 1) * NFB)
                    chg, chv = chains[2 * fb:2 * fb + 2]
                    vm = wk.tile([TARGET, NFB], f32, tag="z", bufs=2,
                                 name=f"vm_{fb}")
                    nc.vector.scalar_tensor_tensor(
                        vm[:], chv["p4"][:], r2b3_sb[:], mb_sb[:, fsl],
                        op0=ALU.add, op1=ALU.mult)
                    gate_s = wk.tile([TARGET, NFB], f32, tag="r", bufs=2,
                                     name=f"gate_{fb}")
                    nc.scalar.activation(gate_s[:], chg["p4"][:], AF.Sigmoid,
                                         bias=r1b3_sb[:])
                    for gg in range(GPB):
                        ga = fb * GPB + gg
                        sc = wk.tile([TARGET, NR], f32, tag="t1", bufs=2,
                                     name=f"sc_{fb}_{gg}")
                        nc.vector.scalar_tensor_tensor(
                            sc[:], gate_s[:, gg * NR:(gg + 1) * NR], 1.0,
                            vm[:, gg * NR:(gg + 1) * NR],
                            op0=ALU.mult, op1=ALU.mult,
                            accum_out=out_sb[:, ga:ga + 1])

                # ---- message passing steps ----
                for s in range(NSTEP):
                    hcs = [0] if s == 0 else list(range(HC))

                    # -- phase A (per graph): projections + agg -> mT_all --
                    def proj_lhsT(g, hc):
                        if s == 0:
                            assert hc == 0
                            if GPB <= g < GPB + 2:
                                return boot_hT0[g - GPB]
                            return hT0_sb[:, g, :]
                        return hT_all[:, hc, g * NR:(g + 1) * NR]

                    def projections(g):
                        P_sb = []
                        cpeng = [nc.vector.tensor_copy, nc.scalar.copy,
                                 nc.scalar.copy, nc.scalar.copy]
                        if s == 0:
                            cpeng[2] = nc.vector.tensor_copy
                        for l in range(L):
                            # s0: spread projection PSUM across pP + the
                            # (GRU-idle) pG2 pool
                            ptag = "pP" if l % 2 == 0 else "pG2"
                            pp = ps.tile([128, MSG], f32, tag=ptag, bufs=2,
                                         name=f"pp_{s}_{g}_{l}")
                            pv = pp[:] if s == 0 else pp[0:NR, :]
                            if s == 0 and GPB <= g < GPB + 2 and l == 0:
                                nc.tensor.matmul(pv, boot_hT0[g - GPB],
                                                 boot_A0,
                                                 start=True, stop=True)
                            else:
                                for i, hc in enumerate(hcs):
                                    nc.tensor.matmul(pv, proj_lhsT(g, hc),
                                                     A_sb[:, l, hc, :],
                                                     start=(i == 0),
                                                     stop=(i == len(hcs) - 1))
                            psb = wk.tile([128, MSG], f32r, tag="P", bufs=8,
                                          name=f"psb_{s}_{g}_{l}")
                            cpeng[l](psb[:], pp[:])
                            P_sb.append(psb)
                        return P_sb

                    def agg_m(g, P_sb):
                        mp = ps.tile([128, MSG], f32, tag="pMT", bufs=2,
                                     name=f"mp_{s}_{g}")
                        for l in range(L):
                            nc.tensor.matmul(mp[0:NR, :],
                                             mask_sb[:, g, l, 0:NR],
                                             P_sb[l][:],
                                             start=(l == 0), stop=(l == L - 1))
                        m_sb = wk.tile([128, MSG], f16, tag="m", bufs=4,
                                       name=f"m_{s}_{g}")
                        nc.vector.tensor_copy(m_sb[0:NR, :], mp[0:NR, :])
                        return m_sb

                    def mT_make(g, m_sb):
                        # s0: pGN is free until the D-phase; avoids 3-deep
                        # pMT pressure from the delayed-transpose pipeline
                        tp = ps.tile([128, MC, NR], f16,
                                     tag="pGN" if s == 0 else "pMT", bufs=2,
                                     name=f"tp_{s}_{g}")
                        for c in range(MC):
                            nc.tensor.transpose(
                                tp[:, c, :],
                                m_sb[0:NR, c * 128:(c + 1) * 128],
                                ident16_t[0:NR, 0:NR])
                        nc.scalar.copy(mT_all[:, :, g * NR:(g + 1) * NR],
                                       tp[:])

                    # fb1's graphs first every step: D(fb1) then needs only
                    # mid-A mT's, and E(s, fb0) consistently trails into the
                    # next A-phase's fb1 half (no fb-parity switch bubble)
                    gorder = list(range(GPB, G)) + list(range(GPB))
                    if s == 0:
                        # short s0 projections expose the agg->copy->
                        # transpose latency: delay each pair's transposes
                        # until after the next pair's projections
                        pending_m = []
                        for gp in range(G // 2):
                            g0, g1 = gorder[2 * gp], gorder[2 * gp + 1]
                            Ps0 = projections(g0)
                            Ps1 = projections(g1)
                            for g_, m_ in pending_m:
                                mT_make(g_, m_)
                            m0 = agg_m(g0, Ps0)
                            m1 = agg_m(g1, Ps1)
                            pending_m = [(g0, m0), (g1, m1)]
                        for g_, m_ in pending_m:
                            mT_make(g_, m_)
                    else:
                        for gp in range(G // 2):
                            g0, g1 = gorder[2 * gp], gorder[2 * gp + 1]
                            Ps0 = projections(g0)
                            Ps1 = projections(g1)
                            mT_make(g0, agg_m(g0, Ps0))
                            mT_make(g1, agg_m(g1, Ps1))

                    # -- phase D+E (per fb, per h-chunk c): gate GEMMs + GRU --
                    def emit_mms(o, mms):
                        for i, (lh, rh, pm) in enumerate(mms):
                            nc.tensor.matmul(o, lh, rh, start=(i == 0),
                                             stop=(i == len(mms) - 1),
                                             perf_mode=pm)

                    fborder = [1, 0]
                    for fb in fborder:
                        fbsl = slice(fb * NFB, (fb + 1) * NFB)
                        for c in range(HC):
                            # gate chunks: r=c, z=4+c, n(i)=8+c, n(h)=8+c
                            r_ps = ps.tile([128, NFB], f32, tag="pP", bufs=2,
                                           name=f"rps_{s}_{fb}_{c}")
                            z_ps = ps.tile([128, NFB], f32, tag="pG2", bufs=2,
                                           name=f"zps_{s}_{fb}_{c}")
                            gin_ps = ps.tile([128, NFB], f32, tag="pGN",
                                             bufs=2, name=f"gin_{s}_{fb}_{c}")
                            ghn_ps = ps.tile([128, NFB], f32, tag="pMT",
                                             bufs=2, name=f"ghn_{s}_{fb}_{c}")
                            # ghn: gh chunks (+ bias plane) only, no wih
                            ghn_mms = []
                            if s == 0:
                                ghn_mms.append((
                                    s0ghn8_t[:, :, c * 128:(c + 1) * 128],
                                    hT08[:, :, fbsl], DR))
                            else:
                                for c2 in (0, 2):
                                    ghn_mms.append((
                                        whh8T_sb[:, c2:c2 + 2,
                                                 (8 + c) * 128:(9 + c) * 128],
                                        hT8_all[:, s % 2, c2:c2 + 2, fbsl],
                                        DR))
                            def rz_mms(gc):
                                csl = slice(gc * 128, (gc + 1) * 128)
                                mms = []
                                if s == 0:
                                    mms.append((s0rz8_t[:, :, csl],
                                                hT08[:, :, fbsl], DR))
                                else:
                                    for c2 in (0, 2):
                                        mms.append((
                                            whh8T_sb[:, c2:c2 + 2, csl],
                                            hT8_all[:, s % 2, c2:c2 + 2,
                                                    fbsl], DR))
                                for c2 in range(MC):
                                    mms.append((wihT_sb[:, c2, csl],
                                                mT_all[:, c2, fbsl], None))
                                return mms

                            # emit in E's consumption order (r first) so
                            # each chunk's activation chain starts earlier
                            emit_mms(r_ps[:], rz_mms(c))
                            emit_mms(z_ps[:], rz_mms(4 + c))
                            emit_mms(ghn_ps[:], ghn_mms)
                            gin_mms = [(wihT_sb[:, c2,
                                                (8 + c) * 128:(9 + c) * 128],
                                        mT_all[:, c2, fbsl],
                                        None) for c2 in range(MC)]
                            emit_mms(gin_ps[:], gin_mms)

                            # -- E: gate nonlinearities + state update --
                            r_sb = wk.tile([128, NFB], f32, tag="r", bufs=2,
                                           name=f"r_{s}_{fb}_{c}")
                            nc.scalar.activation(r_sb[:], r_ps[:], AF.Sigmoid,
                                                 scale=1.0 / GSC,
                                                 bias=gbias_t[:, c:c + 1])
                            if not (s == 0 and c > 0):
                                z_sb = wk.tile([128, NFB], f32, tag="z",
                                               bufs=2, name=f"z_{s}_{fb}_{c}")
                                nc.scalar.activation(
                                    z_sb[:], z_ps[:], AF.Sigmoid,
                                    scale=1.0 / GSC,
                                    bias=gbias_t[:, 4 + c:5 + c])
                            u_sb = wk.tile([128, NFB], f32, tag="u", bufs=2,
                                           name=f"u_{s}_{fb}_{c}")
                            nc.scalar.activation(
                                u_sb[:], z_ps[:], AF.Sigmoid,
                                scale=-1.0 / GSC,
                                bias=gbias_t[:, 8 + c:9 + c])
                            # rhn16 = (ghn + 16*bhn)*r = 16*r*h_n; npre =
                            # gin + rhn16 = 16*(i_n - bin + r*h_n); the 1/16
                            # folds into tanh's scale, bin into its bias
                            rhn = wk.tile([128, NFB], f32, tag="t1", bufs=2,
                                          name=f"rhn_{s}_{fb}_{c}")
                            nc.vector.scalar_tensor_tensor(
                                rhn[:], ghn_ps[:],
                                gbias_t[:, 16 + c:17 + c], r_sb[:],
                                op0=ALU.add, op1=ALU.mult)
                            npre = wk.tile([128, NFB], f32, tag="t2", bufs=2,
                                           name=f"npre_{s}_{fb}_{c}")
                            nc.vector.tensor_add(npre[:], gin_ps[:], rhn[:])
                            n_sb = wk.tile([128, NFB], f32, tag="n", bufs=2,
                                           name=f"n_{s}_{fb}_{c}")
                            nc.scalar.activation(n_sb[:], npre[:], AF.Tanh,
                                                 scale=1.0 / GSC,
                                                 bias=gbias_t[:, 12 + c:13 + c])
                            # h' = (1-z)*n + z*h. zh is off the critical path
                            # (ready before n) -> Pool; un/h'add gate the
                            # next step's proj lhsT -> keep on DVE
                            hsl = hT_all[:, c, fbsl]
                            if s == 0:
                                if c == 0:
                                    zh = wk.tile([128, NFB], f32, tag="zh",
                                                 bufs=2, name=f"zh_{s}_{fb}")
                                    nc.gpsimd.tensor_mul(
                                        zh[:], z_sb[:],
                                        hT0_sb[:, fb * GPB:(fb + 1) * GPB,
                                               0:NR])
                                    un = wk.tile([128, NFB], f32, tag="un",
                                                 bufs=2, name=f"un_{s}_{fb}")
                                    nc.gpsimd.tensor_mul(un[:], u_sb[:],
                                                         n_sb[:])
                                    nc.gpsimd.tensor_add(hsl, un[:], zh[:])
                                else:
                                    nc.gpsimd.tensor_mul(hsl, u_sb[:],
                                                         n_sb[:])
                            else:
                                zh = wk.tile([128, NFB], f32, tag="zh",
                                             bufs=2, name=f"zh_{s}_{fb}_{c}")
                                nc.gpsimd.tensor_mul(zh[:], z_sb[:], hsl)
                                un = wk.tile([128, NFB], f32, tag="un",
                                             bufs=2, name=f"un_{s}_{fb}_{c}")
                                eng_un = (nc.gpsimd if s == NSTEP - 1
                                          else nc.vector)
                                eng_un.tensor_mul(un[:], u_sb[:], n_sb[:])
                                eng_un.tensor_add(hsl, un[:], zh[:])
                            if s < NSTEP - 1:
                                nc.gpsimd.tensor_scalar_mul(
                                    hT8_all[:, (s + 1) % 2, c, fbsl],
                                    hsl, 1.0)
                            if s == NSTEP - 1 and fb == fborder[-1] and c >= 2:
                                # fb1's hT is done (its D/E ran first): start
                                # its readout L0 between fb0's D chunks so
                                # the relus queue ahead of fb0's E tail
                                ro_l0(chains[2 + (c - 2)], 2 + (c - 2))

                        if s == NSTEP - 1 and fb == fborder[-1]:
                            # fb1's hT is complete (its D/E ran first):
                            # overlap its readout L0..L2 with fb0's E tail
                            ro_l1(1)
                            ro_l2(1)

                # zippered readout, fb1 first
                ro_l0(chains[0], 0)
                ro_l0(chains[1], 1)
                ro_l1(0)
                ro_l3(1)
                ro_l2(0)
                ro_finals(1)
                ro_l3(0)
                ro_finals(0)
                nc.sync.dma_start(d_out[:], out_sb[:])

    nc.compile()
    return nc


def _prep_core_inputs(core, g_, h_in, e):
    cs = slice(core * G, (core + 1) * G)
    f = np.float32
    hT0 = np.ascontiguousarray(h_in[cs].transpose(2, 0, 1))  # [F, G, N]
    labels = np.arange(1, L + 1, dtype=f)
    # mask[w, g, l, v] = (e[g, v, w] == l+1)
    e_c = e[cs]  # [G, V, W]
    oh = (e_c[:, None, :, :] == labels[None, :, None, None]).astype(f)
    mask = np.ascontiguousarray(oh.transpose(3, 0, 1, 2))  # [W, G, L, V]
    if MASK8_DMA:
        import ml_dtypes
        mask = mask.astype(ml_dtypes.float8_e4m3)
    # readout mask broadcast (reference: sum(h0, -1) != 0), packed to 112
    rmask = (h_in[cs].sum(-1) != 0).astype(f)[:, 0:NR]      # [G, NR]
    mb = np.broadcast_to(rmask.reshape(1, G * NR),
                         (TARGET, G * NR)).copy()
    return {
        "hT0": hT0,
        "mask": mask,
        "mb": mb,
    }


def _prep_shared_inputs(A, gru_Wih, gru_Whh, gru_bih, gru_bhh,
                        r1_Ws, r1_bs, r2_Ws, r2_bs):
    f = np.float32

    def chunk_rows(M, nch):  # [K, C] -> [128, nch, C] with K = nch*128
        K, C = M.shape
        assert K == nch * 128
        return np.ascontiguousarray(M.reshape(nch, 128, C).transpose(1, 0, 2))

    import ml_dtypes
    f8np = ml_dtypes.float8_e4m3
    GSCf = np.float32(GSC)

    A_t = np.ascontiguousarray(
        A.reshape(L, HC, 128, MSG).transpose(2, 0, 1, 3))  # [128, L, HC, MSG]

    # transposed GRU weights: [feat, chunk, gate-col], premultiplied by GSC
    def t_weights(W, nch):  # W [3H, K] -> [128(f), nch, GC*128]
        Wt = (W * GSCf).reshape(GC, 128, nch, 128)  # [gc, j, kc, f]
        return np.ascontiguousarray(
            Wt.transpose(3, 2, 0, 1).reshape(128, nch, GC * 128))

    wihT = t_weights(np.asarray(gru_Wih, f), MC)
    whh8T_f = t_weights(np.asarray(gru_Whh, f), HC)
    whh8T = whh8T_f.astype(f8np)

    bih = np.asarray(gru_bih, f)
    bhh = np.asarray(gru_bhh, f)
    brz = (bih + bhh)[:2 * H]
    bin_ = bih[2 * H:]
    bhn = bhh[2 * H:]
    # gbias columns: r(0:4) z(4:8) -z(8:12) bin(12:16) bhn*GSC(16:20)
    gbias = np.zeros((128, 20), f)
    for c in range(4):
        gbias[:, c] = brz[c * 128:(c + 1) * 128]
        gbias[:, 4 + c] = brz[H + c * 128:H + (c + 1) * 128]
        gbias[:, 8 + c] = -brz[H + c * 128:H + (c + 1) * 128]
        gbias[:, 12 + c] = bin_[c * 128:(c + 1) * 128]
        gbias[:, 16 + c] = GSCf * bhn[c * 128:(c + 1) * 128]
    # step-0 lhsT pairs
    s0rz8 = np.zeros((128, 2, 2 * H), f)
    s0rz8[:, 0, :] = whh8T_f[:, 0, 0:2 * H]
    s0ghn8 = np.zeros((128, 2, H), f)
    s0ghn8[:, 0, :] = whh8T_f[:, 0, 2 * H:3 * H]

    # readout weights, transposed layout
    r1w0t = np.ascontiguousarray(r1_Ws[0].T)  # [2H, 128]
    r1w0 = np.zeros((128, 5, 128), f)
    for kc in range(4):
        r1w0[:, kc, :] = r1w0t[kc * 128:(kc + 1) * 128]
    r1w0[:, 4, :] = r1w0t[H:H + F_IN]  # h0 chunk (features 0:128 of h0 half)
    r1w1 = np.ascontiguousarray(r1_Ws[1].T.reshape(128, 2, 128))
    r1w2 = chunk_rows(np.ascontiguousarray(r1_Ws[2].T), 2)
    r1w3 = np.ascontiguousarray(r1_Ws[3].T)  # [128, 12]
    r2w0 = chunk_rows(np.ascontiguousarray(r2_Ws[0].T), 4)
    r2w1 = np.ascontiguousarray(r2_Ws[1].T.reshape(128, 2, 128))
    r2w2 = chunk_rows(np.ascontiguousarray(r2_Ws[2].T), 2)
    r2w3 = np.ascontiguousarray(r2_Ws[3].T)

    identcol = np.concatenate([np.eye(128, dtype=f), np.ones((128, 1), f)], 1)
    rowb = np.concatenate([r1w0, r1w1, r1w2, r2w0, r2w1, r2w2], axis=1)
    row3 = np.stack([r1w3, r2w3], axis=1)
    robias = np.concatenate([
        r1_bs[0].reshape(-1, 1).astype(f),
        np.ascontiguousarray(r1_bs[1].reshape(2, 128).T),
        r1_bs[2].reshape(-1, 1).astype(f),
        r2_bs[0].reshape(-1, 1).astype(f),
        np.ascontiguousarray(r2_bs[1].reshape(2, 128).T),
        r2_bs[2].reshape(-1, 1).astype(f)], axis=1)
    rob12 = np.concatenate([r1_bs[3].reshape(-1, 1).astype(f),
                            r2_bs[3].reshape(-1, 1).astype(f)], axis=1)
    return {
        "A": A_t,
        "wihT": wihT,
        "whh8T": np.ascontiguousarray(whh8T),
        "s0rz8": s0rz8.astype(f8np),
        "s0ghn8": s0ghn8.astype(f8np),
        "gbias": gbias,
        "identcol": np.ascontiguousarray(identcol),
        "rowb": np.ascontiguousarray(rowb),
        "row3": np.ascontiguousarray(row3),
        "robias": np.ascontiguousarray(robias),
        "rob12": np.ascontiguousarray(rob12),
    }


def _get_nc(nreps=1):
    key = ("nc", nreps)
    if key not in _CACHE:
        _CACHE[key] = _build(nreps)
    return _CACHE[key]


def _run(in_maps, **kwargs):
    nc = _get_nc()
    return run_bass_kernel_spmd(nc, in_maps, core_ids=list(range(NCORES)),
                                **kwargs)


def make_in_maps(g, h_in, e, A, gru_Wih, gru_Whh, gru_bih, gru_bhh,
                 r1_W0, r1_b0, r1_W1, r1_b1, r1_W2, r1_b2, r1_W3, r1_b3,
                 r2_W0, r2_b0, r2_W1, r2_b1, r2_W2, r2_b2, r2_W3, r2_b3):
    r1_Ws, r1_bs = [r1_W0, r1_W1, r1_W2, r1_W3], [r1_b0, r1_b1, r1_b2, r1_b3]
    r2_Ws, r2_bs = [r2_W0, r2_W1, r2_W2, r2_W3], [r2_b0, r2_b1, r2_b2, r2_b3]
    arrs = {k: np.asarray(v, np.float32) for k, v in dict(
        g=g, h_in=h_in, e=e, A=A, gru_Wih=gru_Wih, gru_Whh=gru_Whh,
        gru_bih=gru_bih, gru_bhh=gru_bhh).items()}
    r1_Ws = [np.asarray(w, np.float32) for w in r1_Ws]
    r1_bs = [np.asarray(b, np.float32) for b in r1_bs]
    r2_Ws = [np.asarray(w, np.float32) for w in r2_Ws]
    r2_bs = [np.asarray(b, np.float32) for b in r2_bs]
    shared = _prep_shared_inputs(arrs["A"], arrs["gru_Wih"], arrs["gru_Whh"],
                                 arrs["gru_bih"], arrs["gru_bhh"],
                                 r1_Ws, r1_bs, r2_Ws, r2_bs)
    f = np.float32
    in_maps = []
    for core in range(NCORES):
        m = dict(shared)
        m.update(_prep_core_inputs(core, arrs["g"], arrs["h_in"], arrs["e"]))
        boot = np.concatenate([m["hT0"][:, GPB, :], m["hT0"][:, GPB + 1, :],
                               np.asarray(arrs["A"][0, 0:128, :], f)], 1)
        m["boot"] = np.ascontiguousarray(boot)
        in_maps.append(m)
    return in_maps


def kernel(**inputs):
    in_maps = make_in_maps(**inputs)
    res = _run(in_maps)
    out = np.zeros((B, TARGET), np.float32)
    for core in range(NCORES):
        out[core * G:(core + 1) * G] = res.results[core]["out"].T
    return out


if __name__ == "__main__":
    import reference
    inputs = {k: np.asarray(v) for k, v in reference.setup_inputs().items()}
    expected = np.asarray(reference.reference(**inputs))
    actual = kernel(**inputs)
    scale = np.abs(expected).max()
    err = np.abs(actual - expected).max() / scale
    print("Relative error:", err)
